# revision 1
# baseline (speedup 1.0000x reference)
"""Trainium2 Bass kernel for ExampleGuidedAttention (N=8, C=256, H=W=64).

Data-parallel over batch N across 8 NeuronCores; each core computes one
batch element's full guided attention.

Algorithm notes (per core):
  q = conv_w @ src_pix                      [64, 4096]   (PE, bf16)
  S^T[j,i] = sum_o q[o,j] q[o,i]            (PE, bf16; S symmetric; two
             j-blocks packed in the 128x128 array via tile_position
             (row groups 0-63 / 64-127) since the contraction is only 64)
  F[j,i] = exp(S^T[j,i] - 64)               (ACT; global shift keeps fp32
             exp in range -- softmax ratio unchanged; diag scores are
             chi2(64) so they reach ~120)
  Z[p]   = sum_i F (free-dim reduce of the symmetric tiles on DVE)
  O[c,i] = sum_j pixT[j,c] * F[j,i]         (PE, bf16, natural layout)
  out    = [ (1-m)*ref_att*invZ + m*ref ; src_att*invZ ]

Finalize for slices 0..6 runs on DVE while the PE is still doing slice
7's apply matmuls; slice 7's copy-out doubles as its normalize.
"""

import numpy as np

import concourse.bass as bass
import concourse.mybir as mybir
import concourse.tile as tile
from concourse import bacc, bass_utils
from concourse.bass import ts
from concourse.masks import make_identity

P = 128
C = 256          # feature channels
CQ = 64          # query channels
HW = 4096        # pixels per image
NB = HW // P     # 32 pixel blocks (contraction chunks)
SLICE = 512
NS = HW // SLICE  # 8 output column slices
NCORES = 8

F32 = mybir.dt.float32
BF16 = mybir.dt.bfloat16
EXP = mybir.ActivationFunctionType.Exp
AX_X = mybir.AxisListType.X


def _build_body(tc, src, ref, mask, wT, out):
    nc = tc.nc
    src_r = src.ap().rearrange("(ci p) j -> p ci j", p=P)   # [128, 2, 4096]
    ref_r = ref.ap().rearrange("(ci p) j -> p ci j", p=P)
    wT_r = wT.ap().rearrange("(ci p) o -> p ci o", p=P)     # [128, 2, 64]
    out_r = out.ap().rearrange("(cb p) j -> cb p j", p=P)   # [4, 128, 4096]

    with (
        tc.tile_pool(name="persist", bufs=1) as persist,
        tc.tile_pool(name="ps_s", bufs=4, space="PSUM") as ps_s,
        tc.tile_pool(name="ps_o", bufs=4, space="PSUM") as ps_o,
        tc.tile_pool(name="dram", bufs=1, space="DRAM") as dram,
    ):
        # bf16 ref copy doubles as the blend operand (saves the fp32 copy)
        refb = persist.tile([P, 2, HW], BF16)
        # q duplicated into both partition halves so scores matmuls can be
        # row-packed: tile at rows 0-63 and rows 64-127 run concurrently.
        q2 = persist.tile([P, HW], BF16)
        pixT_src = persist.tile([P, NB, C], BF16)
        pixT_ref = persist.tile([P, NB, C], BF16)
        wT_sb = persist.tile([P, 2, CQ], BF16)
        zpart = persist.tile([P, NB, NS], F32)
        z_all = persist.tile([P, NB], F32)
        invz = persist.tile([P, NB], F32)
        mask_rep = persist.tile([P, HW], F32)
        invz_rep = persist.tile([P, HW], F32)
        exp_bias = persist.tile([P, 1], F32)
        ident = persist.tile([P, P], F32)
        invz_T = persist.tile([NB, P], F32)
        zrow = dram.tile([HW], F32)
        nc.vector.memset(exp_bias, -64.0)
        make_identity(nc, ident)

        nc.sync.dma_start(out=wT_sb, in_=wT_r)
        for s in range(NS):
            nc.scalar.dma_start(
                out=mask_rep[:, ts(s, SLICE)],
                in_=mask.ap()[ts(s, SLICE)].partition_broadcast(P),
            )

        with tc.tile_pool(name="early", bufs=1) as early:
            # PE warmup: back-to-back matmuls on zeroed data latch the HAM
            # clock gate to 8/8 (2.4 GHz) while input DMAs stream in.
            warm_sb = early.tile([P, SLICE], BF16)
            nc.vector.memset(warm_sb, 0.0)
            warm_ps = ps_s.tile([P, SLICE], F32, name="warm_ps", tag="pss")
            for _ in range(18):
                nc.tensor.matmul(
                    warm_ps, warm_sb[:, 0:P], warm_sb, start=True, stop=True
                )
            srcb = early.tile([P, 2, HW], BF16)
            # src casts first (conv + src transpose depend on them)
            for ci in range(2):
                for s in range(NS):
                    sl = ts(s, SLICE)
                    nc.gpsimd.dma_start(out=srcb[:, ci, sl], in_=src_r[:, ci, sl])
            for ci in range(2):
                for s in range(NS):
                    sl = ts(s, SLICE)
                    nc.gpsimd.dma_start(out=refb[:, ci, sl], in_=ref_r[:, ci, sl])
            # XBAR transposes on two HWDGE queues, split in j-halves so each
            # can start as soon as half the casts have landed:
            # pixT[p, b, c] = pix[c, b*128+p]
            for ci in range(2):
                cs = slice(ci * P, (ci + 1) * P)
                for h in range(2):
                    jh = slice(h * (HW // 2), (h + 1) * (HW // 2))
                    bh = slice(h * (NB // 2), (h + 1) * (NB // 2))
                    nc.sync.dma_start_transpose(
                        out=pixT_src[:, bh, cs], in_=srcb[:, ci, jh]
                    )
                    nc.scalar.dma_start_transpose(
                        out=pixT_ref[:, bh, cs], in_=refb[:, ci, jh]
                    )
            # 1x1 conv: q = wT.T @ src_pix; write q into both partition halves
            for s in range(NS):
                sl = ts(s, SLICE)
                psq = ps_s.tile([CQ, SLICE], F32, name="psq", tag="pss")
                for ci in range(2):
                    nc.tensor.matmul(
                        psq,
                        wT_sb[:, ci, :],
                        srcb[:, ci, sl],
                        start=(ci == 0),
                        stop=(ci == 1),
                    )
                nc.vector.tensor_copy(out=q2[0:CQ, sl], in_=psq)
                nc.vector.tensor_copy(out=q2[CQ:P, sl], in_=psq)

        def scores_and_exp(s, f_sb):
            sl = ts(s, SLICE)
            for jp in range(NB // 2):
                jb0, jb1 = 2 * jp, 2 * jp + 1
                pss0 = ps_s.tile([P, SLICE], F32, name="pss0", tag="pss")
                pss1 = ps_s.tile([P, SLICE], F32, name="pss1", tag="pss")
                nc.tensor.matmul(
                    pss0, q2[0:CQ, ts(jb0, P)], q2[0:CQ, sl],
                    start=True, stop=True, tile_position=(0, 0),
                )
                nc.tensor.matmul(
                    pss1, q2[CQ:P, ts(jb1, P)], q2[CQ:P, sl],
                    start=True, stop=True, tile_position=(CQ, 0),
                )
                for jb, pss in ((jb0, pss0), (jb1, pss1)):
                    nc.scalar.activation(
                        out=f_sb[:, jb, :], in_=pss, func=EXP, bias=exp_bias
                    )
            # reduce in groups of 8 j-blocks: big enough to amortize DVE op
            # overhead, small enough not to serialize the DVE queue
            for g in range(NB // 8):
                nc.vector.reduce_sum(
                    out=zpart[:, ts(g, 8), s : s + 1],
                    in_=f_sb[:, ts(g, 8), :],
                    axis=AX_X,
                )

        def apply_mm(s, f_sb):
            psos = [
                ps_o.tile([P, SLICE], F32, name=f"pso{cb}", tag="pso")
                for cb in range(4)
            ]
            for jb in range(NB):
                for cb in range(4):
                    pt = pixT_src if cb < 2 else pixT_ref
                    lhs = pt[:, jb, (cb % 2) * P : (cb % 2 + 1) * P]
                    nc.tensor.matmul(
                        psos[cb], lhs, f_sb[:, jb, :],
                        start=(jb == 0), stop=(jb == NB - 1),
                    )
            return psos

        def copy_out(s, psos):
            sl = ts(s, SLICE)
            for cb in range(4):
                nc.vector.tensor_copy(out=o_sb[:, cb, sl], in_=psos[cb])

        def finalize(lo, hi, dma_engines, skip_norm=False):
            """Normalize + blend + store for pixel columns [lo:hi).

            All elementwise work stays on DVE: GpSimd shares (and locks) the
            DVE SBUF port, so splitting across both engines makes each ~3x
            slower with no net gain.
            """
            r = slice(lo, hi)
            if not skip_norm:
                for cb in range(4):
                    nc.vector.tensor_mul(
                        o_sb[:, cb, r], o_sb[:, cb, r], invz_rep[:, r]
                    )
            for cb in (2, 3):
                ci = cb - 2
                nc.vector.tensor_sub(tmp[:, r], refb[:, ci, r], o_sb[:, cb, r])
                nc.vector.tensor_mul(tmp[:, r], tmp[:, r], mask_rep[:, r])
                nc.vector.tensor_add(o_sb[:, cb, r], o_sb[:, cb, r], tmp[:, r])
            # out rows: [flow(=cb2,3), src_att(=cb0,1)]
            for k, cb in enumerate([2, 3, 0, 1]):
                eng = dma_engines[k % len(dma_engines)]
                for s in range(lo // SLICE, (hi + SLICE - 1) // SLICE):
                    sl = ts(s, SLICE)
                    eng.dma_start(out=out_r[k, :, sl], in_=o_sb[:, cb, sl])

        with tc.tile_pool(name="obuf", bufs=1) as obuf:
            o_sb = obuf.tile([P, 4, HW], F32)
            tmp = obuf.tile([P, HW], F32)
            with tc.tile_pool(name="fbuf", bufs=1) as fbuf:
                for s in range(NS - 1):
                    f_sb = fbuf.tile([P, NB, SLICE], BF16, name="f_sb", tag="f")
                    scores_and_exp(s, f_sb)
                    psos = apply_mm(s, f_sb)
                    copy_out(s, psos)
                # slice 7: scores/exp first so Z completes, then overlap
                # finalize(0..6) with slice 7's apply matmuls
                s7 = NS - 1
                f_sb7 = fbuf.tile([P, NB, SLICE], BF16, name="f_sb", tag="f")
                scores_and_exp(s7, f_sb7)
                nc.vector.reduce_sum(out=z_all, in_=zpart, axis=AX_X)
                nc.vector.reciprocal(out=invz, in_=z_all)
                # broadcast invz[pixel] across the 128 channel partitions:
                # PE-transpose [128p, 32b] -> [32b, 128p], bounce through
                # DRAM contiguously, then partition-broadcast back (a direct
                # scatter of the untransposed layout costs ~16us)
                ps_t = ps_s.tile([NB, P], F32, name="ps_t", tag="pss")
                nc.tensor.transpose(ps_t, invz[:, :], ident)
                nc.vector.tensor_copy(out=invz_T, in_=ps_t)
                nc.sync.dma_start(
                    out=zrow.rearrange("(b q) -> b q", q=P), in_=invz_T
                )
                for s in range(NS):
                    sl = ts(s, SLICE)
                    nc.sync.dma_start(
                        out=invz_rep[:, sl], in_=zrow[sl].partition_broadcast(P)
                    )
                psos7 = apply_mm(s7, f_sb7)
                finalize(0, (NS - 1) * SLICE, [nc.sync, nc.scalar])
                # slice-7 copy-out doubles as its normalize (Z is ready now)
                sl7 = ts(s7, SLICE)
                for cb in range(4):
                    nc.vector.tensor_mul(
                        o_sb[:, cb, sl7], psos7[cb], invz_rep[:, sl7]
                    )
                finalize((NS - 1) * SLICE, HW, [nc.scalar, nc.sync], skip_norm=True)


def build():
    nc = bacc.Bacc(
        "TRN2",
        target_bir_lowering=False,
        debug=False,
        enable_asserts=False,
        num_devices=NCORES,
    )
    src = nc.dram_tensor("src", (C, HW), F32, kind="ExternalInput")
    ref = nc.dram_tensor("ref", (C, HW), F32, kind="ExternalInput")
    mask = nc.dram_tensor("mask", (HW,), F32, kind="ExternalInput")
    wT = nc.dram_tensor("wT", (C, CQ), BF16, kind="ExternalInput")
    out = nc.dram_tensor("out", (2 * C, HW), F32, kind="ExternalOutput")
    with tile.TileContext(nc) as tc:
        _build_body(tc, src, ref, mask, wT, out)
    nc.compile()
    return nc


_CACHE = {}


def _get_nc():
    if "nc" not in _CACHE:
        _CACHE["nc"] = build()
    return _CACHE["nc"]


def _in_maps(src_mask, src_feature, ref_feature, conv_w):
    import ml_dtypes

    n_batch = src_feature.shape[0]
    wT = np.ascontiguousarray(
        np.asarray(conv_w, dtype=np.float32).T.astype(ml_dtypes.bfloat16)
    )
    maps = []
    for n in range(n_batch):
        maps.append(
            {
                "src": np.ascontiguousarray(
                    np.asarray(src_feature[n], dtype=np.float32).reshape(C, HW)
                ),
                "ref": np.ascontiguousarray(
                    np.asarray(ref_feature[n], dtype=np.float32).reshape(C, HW)
                ),
                "mask": np.ascontiguousarray(
                    np.asarray(src_mask[n], dtype=np.float32).reshape(HW)
                ),
                "wT": wT,
            }
        )
    return maps


def _install_ntff_hook():
    """The agent image's antenv lacks axon_hooks; recreate it so
    run_bass_kernel_spmd(trace=True) can capture NTFF profiles."""
    import sys
    import types

    if "antenv.axon_hooks" in sys.modules:
        return
    import antenv
    from trn_agent_boot.trn_boot import _ntff_profile_via_ctypes

    hook = _ntff_profile_via_ctypes("/opt/axon/libaxon_pjrt.so")
    mod = types.ModuleType("antenv.axon_hooks")
    mod._hook = hook
    mod.set_axon_ntff_profile_hook = lambda h: setattr(mod, "_hook", h)
    mod.get_axon_ntff_profile_hook = lambda: mod._hook
    sys.modules["antenv.axon_hooks"] = mod
    antenv.axon_hooks = mod


def run(src_mask, src_feature, ref_feature, conv_w, trace=False):
    """Run on 8 NeuronCores. Returns (output [N,2C,H,W], BassKernelResults)."""
    n_batch, c, h, w = src_feature.shape
    if trace:
        _install_ntff_hook()
    nc = _get_nc()
    maps = _in_maps(src_mask, src_feature, ref_feature, conv_w)
    res = bass_utils.run_bass_kernel_spmd(
        nc, maps, core_ids=list(range(NCORES)), trace=trace
    )
    out = np.stack([r["out"] for r in res.results], axis=0)
    return out.reshape(n_batch, 2 * c, h, w).astype(np.float32), res


def kernel(src_mask, src_feature, ref_feature, conv_w):
    out, _ = run(src_mask, src_feature, ref_feature, conv_w)
    return out



# revision 11
# speedup vs baseline: 1.4331x; 1.4331x over previous
"""Trainium2 Bass kernel for ExampleGuidedAttention (N=8, C=256, H=W=64).

Data-parallel over batch N across 8 NeuronCores; each core computes one
batch element's full guided attention.

Algorithm notes (per core):
  q = conv_w @ src_pix                      [64, 4096]   (PE, bf16)
  S^T[j,i] = sum_o q[o,j] q[o,i]            (PE, bf16; S symmetric; two
             j-blocks packed in the 128x128 array via tile_position)
  F[j,i] = exp(S^T[j,i] - 64 + 20*ln2)      (ACT; global shift keeps the
             fp32 exp in range; the 2^20 factor cancels against 1/Z)
  Per column-slice s (512 pixels) the F tiles span ALL j, so
  Z[i] = sum_j F[j,i] is computed per-slice with ones-vector matmuls on
  the PE, and each slice normalizes + blends + stores immediately --
  no full-image unnormalized buffer and no end-of-kernel fixup tail.

  Off-diagonal j-block tiles (28 of 32 per slice) are stored in fp8-e4m3
  and applied with DoubleRow matmuls (2 j-blocks per pass, 2x PE rate).
  The 4 diagonal-crossing tiles stay bf16 so the dominant near-diagonal
  attention terms keep full precision; all tiles share the 2^20 scale so
  they accumulate consistently in PSUM and the scale cancels in 1/Z.

  out = [ (1-m)*ref_att*invZ + m*ref ; src_att*invZ ]

The issue order software-pipelines slice s's scores/exp chain (ACT
bound, single PSUM group buffer) against slice s-1's apply matmuls so
the PE never stalls on the exp chain.
"""

import math

import numpy as np

import concourse.bass as bass
import concourse.mybir as mybir
import concourse.tile as tile
from concourse import bacc, bass_utils
from concourse.bass import ts
from concourse.alu_op_type import AluOpType

P = 128
C = 256          # feature channels
CQ = 64          # query channels
HW = 4096        # pixels per image
NB = HW // P     # 32 pixel blocks (contraction chunks)
SLICE = 512
NS = HW // SLICE  # 8 output column slices
NG = NB // 4      # 8 score groups of 4 j-blocks per slice
NCORES = 8
K_SCALE = 13.0    # F scaled by 2^13: off-diag fp8 overflow headroom to S=70

F32 = mybir.dt.float32
BF16 = mybir.dt.bfloat16
F8 = mybir.dt.float8e4
EXP = mybir.ActivationFunctionType.Exp
DR = mybir.MatmulPerfMode.DoubleRow


def _build_body(tc, src, ref, mask, wT, out):
    nc = tc.nc
    src_r = src.ap().rearrange("(ci p) j -> p ci j", p=P)   # [128, 2, 4096]
    ref_r = ref.ap().rearrange("(ci p) j -> p ci j", p=P)
    wT_r = wT.ap().rearrange("(ci p) o -> p ci o", p=P)     # [128, 2, 64]
    out_r = out.ap().rearrange("(cb p) j -> cb p j", p=P)   # [4, 128, 4096]

    with (
        tc.tile_pool(name="persist", bufs=1) as persist,
        tc.tile_pool(name="ps_sc", bufs=1, space="PSUM") as ps_sc,
        tc.tile_pool(name="ps_ap", bufs=1, space="PSUM") as ps_ap,
        tc.tile_pool(name="ps_z", bufs=1, space="PSUM") as ps_z,
        tc.tile_pool(name="dram", bufs=1, space="DRAM") as dram,
    ):
        refb = persist.tile([P, 2, HW], BF16)
        q2 = persist.tile([P, HW], BF16)
        pixT_bf = persist.tile([P, NB, 2 * C], BF16)   # [src 256 | ref 256]
        pixT_f8 = persist.tile([P, NB, 2 * C], F8)
        wT_sb = persist.tile([P, 2, CQ], BF16)
        mask_rep = persist.tile([P, HW], BF16)
        omask_rep = persist.tile([P, HW], BF16)        # 1 - mask
        mref = persist.tile([P, 2, HW], BF16)          # mask * ref
        exp_bias = persist.tile([P, 1], F32)
        ones8 = persist.tile([P, 2, 16], F8)  # 16B k-pair stride for dual-fp8 ldweights
        ones_bf = persist.tile([P, 2], BF16)
        warm_sb = persist.tile([P, SLICE], BF16)
        zrow = dram.tile([NS, SLICE], F32)
        nc.vector.memset(exp_bias, -64.0 + K_SCALE * math.log(2.0))
        nc.vector.memset(ones8, 1.0)
        nc.vector.memset(ones_bf, 1.0)
        nc.vector.memset(warm_sb, 0.0)

        nc.sync.dma_start(out=wT_sb, in_=wT_r)
        for s in range(NS):
            nc.gpsimd.dma_start(
                out=mask_rep[:, ts(s, SLICE)],
                in_=mask.ap()[ts(s, SLICE)].partition_broadcast(P),
            )

        with tc.tile_pool(name="early", bufs=4) as early, \
             tc.tile_pool(name="early1", bufs=1) as early1, \
             tc.tile_pool(name="ps_cv", bufs=1, space="PSUM") as ps_cv:
            # PE warmup: back-to-back matmuls on zeroed data keep the HAM
            # clock gate at 8/8 (2.4 GHz) while input DMAs stream in.
            warm_ps = ps_sc.tile([P, 4, SLICE], F32, name="pss", tag="pss")
            for r in range(40):
                nc.tensor.matmul(
                    warm_ps[:, r % 4, :], warm_sb[:, 0:P], warm_sb,
                    start=True, stop=True,
                )
            # warm the ACT exp table so slice 0 doesn't pay the table load
            nc.scalar.activation(
                out=warm_sb[:, 0:1], in_=warm_sb[:, 0:1], func=EXP,
                bias=exp_bias,
            )

            srcb = early1.tile([P, 2, HW], BF16)
            # fp32 inputs via fast hardware DMA.  src casts on DVE (conv ->
            # q gates everything); ref casts on ACT so the DVE can proceed
            # to q2 / fp8 work without head-of-line blocking.
            src_stages = []
            for s in range(NS):
                st = early.tile([P, 2, SLICE], F32, name="stage", tag="st")
                (nc.sync if s % 2 == 0 else nc.scalar).dma_start(
                    out=st, in_=src_r[:, :, ts(s, SLICE)]
                )
                src_stages.append(st)
            ref_stages = []
            for s in range(NS):
                st = early.tile([P, 2, SLICE], F32, name="rstage", tag="rst")
                nc.gpsimd.dma_start(out=st, in_=ref_r[:, :, ts(s, SLICE)])
                ref_stages.append(st)
            for s in range(NS):
                nc.vector.tensor_copy(
                    out=srcb[:, :, ts(s, SLICE)], in_=src_stages[s]
                )
            # 1x1 conv: q = wT.T @ src_pix; q into both partition halves
            for s in range(NS):
                sl = ts(s, SLICE)
                psq = ps_cv.tile([CQ, SLICE], F32, name="psq", tag="psq")
                for ci in range(2):
                    nc.tensor.matmul(
                        psq,
                        wT_sb[:, ci, :],
                        srcb[:, ci, sl],
                        start=(ci == 0),
                        stop=(ci == 1),
                    )
                nc.vector.tensor_copy(out=q2[0:CQ, sl], in_=psq)
                nc.vector.tensor_copy(out=q2[CQ:P, sl], in_=psq)
            for s in range(NS):
                nc.scalar.activation(
                    out=refb[:, :, ts(s, SLICE)], in_=ref_stages[s],
                    func=mybir.ActivationFunctionType.Copy,
                )

            # XBAR transposes: pixT[p, b, c] = pix[c, b*128+p]; j-halves so
            # each starts as soon as half the casts have landed.
            for ci in range(2):
                cs = slice(ci * P, (ci + 1) * P)
                cs_r = slice(C + ci * P, C + (ci + 1) * P)
                for h in range(2):
                    jh = slice(h * (HW // 2), (h + 1) * (HW // 2))
                    bh = slice(h * (NB // 2), (h + 1) * (NB // 2))
                    nc.sync.dma_start_transpose(
                        out=pixT_bf[:, bh, cs], in_=srcb[:, ci, jh]
                    )
                    nc.scalar.dma_start_transpose(
                        out=pixT_bf[:, bh, cs_r], in_=refb[:, ci, jh]
                    )
            # fp8 copy of the pixel transpose for DoubleRow matmuls; src
            # column half first, j-ascending, so apply wave A can start
            # before the ref transposes have even landed.
            for b in range(0, NB, 4):
                nc.vector.tensor_copy(
                    out=pixT_f8[:, b : b + 4, 0:C],
                    in_=pixT_bf[:, b : b + 4, 0:C],
                )
            for b in range(0, NB, 4):
                nc.vector.tensor_copy(
                    out=pixT_f8[:, b : b + 4, C : 2 * C],
                    in_=pixT_bf[:, b : b + 4, C : 2 * C],
                )
            # blend precomputes: 1-m and m*ref
            nc.vector.tensor_scalar(
                out=omask_rep, in0=mask_rep, scalar1=-1.0, scalar2=1.0,
                op0=AluOpType.mult, op1=AluOpType.add,
            )
            for ci in range(2):
                nc.vector.tensor_mul(mref[:, ci, :], mask_rep, refb[:, ci, :])

        def f8slot(s, jb):
            return jb if jb < 4 * s else jb - 4

        with tc.tile_pool(name="fbuf", bufs=2) as fbuf, \
             tc.tile_pool(name="obuf", bufs=2) as obuf, \
             tc.tile_pool(name="zbuf", bufs=2) as zbuf:

            def emit_scores_group(s, g, f8, fbf):
                sl = ts(s, SLICE)
                pss = ps_sc.tile([P, 4, SLICE], F32, name="pss", tag="pss")
                for jp in range(2):
                    jb0, jb1 = 4 * g + 2 * jp, 4 * g + 2 * jp + 1
                    nc.tensor.matmul(
                        pss[:, 2 * jp, :], q2[0:CQ, ts(jb0, P)], q2[0:CQ, sl],
                        start=True, stop=True, tile_position=(0, 0),
                    )
                    nc.tensor.matmul(
                        pss[:, 2 * jp + 1, :], q2[CQ:P, ts(jb1, P)],
                        q2[CQ:P, sl],
                        start=True, stop=True, tile_position=(CQ, 0),
                    )
                if g == s:
                    nc.scalar.activation(
                        out=fbf, in_=pss, func=EXP, bias=exp_bias
                    )
                else:
                    fs = f8slot(s, 4 * g)
                    nc.scalar.activation(
                        out=f8[:, fs : fs + 4, :], in_=pss, func=EXP,
                        bias=exp_bias,
                    )

            def emit_z(s, f8, fbf):
                zps = ps_z.tile([2, SLICE], F32, name="psz", tag="psz")
                n_z = 0
                for g in range(NG):
                    if g == s:
                        for r in range(4):
                            nc.tensor.matmul(
                                zps, ones_bf, fbf[:, r, :],
                                start=(n_z == 0), stop=(n_z == 17),
                            )
                            n_z += 1
                    else:
                        fs = f8slot(s, 4 * g)
                        for r in range(2):
                            nc.tensor.matmul(
                                zps, ones8[:, :, 0:2],
                                f8[:, fs + 2 * r : fs + 2 * r + 2, :],
                                start=(n_z == 0), stop=(n_z == 17),
                                perf_mode=DR,
                            )
                            n_z += 1
                zinv_row = zbuf.tile([1, SLICE], F32, name="zr", tag="zr")
                invz_rep = zbuf.tile([P, SLICE], F32, name="zrep", tag="zrep")
                nc.vector.reciprocal(out=zinv_row, in_=zps[0:1, :])
                nc.gpsimd.dma_start(out=zrow[s], in_=zinv_row)
                nc.gpsimd.dma_start(
                    out=invz_rep, in_=zrow[s].partition_broadcast(P)
                )
                return invz_rep

            def emit_warm_chunk():
                wz = ps_z.tile([2, SLICE], F32, name="psz", tag="psz")
                for r in range(9):
                    nc.tensor.matmul(
                        wz, warm_sb[:, 0:2], warm_sb,
                        start=(r == 0), stop=(r == 8),
                    )

            def emit_apply_wave(ctx, wave):
                """One wave = 2 output channel blocks accumulated over all
                32 j-blocks: fp8 DoubleRow off-diagonal + bf16 diagonal."""
                s, f8, fbf = ctx["s"], ctx["f8"], ctx["fbf"]
                pso = ps_ap.tile([P, 2, SLICE], F32, name="psa", tag="psa")
                ctx["pso"] = pso
                for half in range(2):
                    cs = slice((2 * wave + half) * P, (2 * wave + half + 1) * P)
                    for g in range(NG):
                        if g == s:
                            for r in range(4):
                                jb = 4 * g + r
                                nc.tensor.matmul(
                                    pso[:, half, :], pixT_bf[:, jb, cs],
                                    fbf[:, r, :],
                                    start=(g == 0 and r == 0),
                                    stop=(g == NG - 1 and r == 3),
                                )
                        else:
                            fs = f8slot(s, 4 * g)
                            for r in range(2):
                                jb = 4 * g + 2 * r
                                nc.tensor.matmul(
                                    pso[:, half, :],
                                    pixT_f8[:, jb : jb + 2, cs],
                                    f8[:, fs + 2 * r : fs + 2 * r + 2, :],
                                    start=(g == 0 and r == 0),
                                    stop=(g == NG - 1 and r == 1),
                                    perf_mode=DR,
                                )

            def emit_finalize_wave(ctx, wave):
                s, invz_rep = ctx["s"], ctx["invz"]
                sl = ts(s, SLICE)
                outb = ctx["outb"]
                scr = obuf.tile([P, 2, SLICE], F32, name="scr", tag="scr")
                nc.vector.tensor_copy(out=scr, in_=ctx["pso"])
                if wave == 0:
                    # src_att * invz -> out rows 256..512
                    for half in range(2):
                        nc.vector.tensor_mul(
                            outb[:, 2 + half, :], scr[:, half, :], invz_rep
                        )
                else:
                    # flow = ref_att*invz*(1-m) + m*ref -> out rows 0..256
                    a_s = obuf.tile([P, SLICE], F32, name="a_s", tag="a_s")
                    nc.vector.tensor_mul(a_s, omask_rep[:, sl], invz_rep)
                    for half in range(2):
                        nc.vector.tensor_mul(
                            outb[:, half, :], scr[:, half, :], a_s
                        )
                        nc.vector.tensor_add(
                            outb[:, half, :], outb[:, half, :],
                            mref[:, half, sl],
                        )
                    oq = [nc.sync, nc.scalar, nc.gpsimd]
                    for k in range(4):
                        oq[(s + k) % 3].dma_start(
                            out=out_r[k, :, sl], in_=outb[:, k, :]
                        )

            prev = None
            for s in range(NS):
                f8 = fbuf.tile([P, NB - 4, SLICE], F8, name="f8", tag="f8")
                fbf = fbuf.tile([P, 4, SLICE], BF16, name="fbf", tag="fbf")
                ctx = {"s": s, "f8": f8, "fbf": fbf}
                ctx["outb"] = None
                # interleave this slice's scores/exp chain with the previous
                # slice's apply matmuls so the PE always has work while the
                # exp chain (single PSUM group buffer) serializes.
                for g in range(NG):
                    emit_scores_group(s, g, f8, fbf)
                    if prev is not None:
                        if g == 0:
                            prev["outb"] = obuf.tile(
                                [P, 4, SLICE], F32, name="outb", tag="outb"
                            )
                            emit_apply_wave(prev, 0)
                        elif g == 3:
                            emit_finalize_wave(prev, 0)
                            emit_apply_wave(prev, 1)
                        elif g == 7:
                            emit_finalize_wave(prev, 1)
                    elif g < 7:
                        emit_warm_chunk()
                ctx["invz"] = emit_z(s, f8, fbf)
                prev = ctx
            # drain: last slice's apply + finalize
            prev["outb"] = obuf.tile([P, 4, SLICE], F32, name="outb", tag="outb")
            emit_apply_wave(prev, 0)
            emit_finalize_wave(prev, 0)
            emit_apply_wave(prev, 1)
            emit_finalize_wave(prev, 1)


def build():
    nc = bacc.Bacc(
        "TRN2",
        target_bir_lowering=False,
        debug=False,
        enable_asserts=False,
        num_devices=NCORES,
    )
    src = nc.dram_tensor("src", (C, HW), F32, kind="ExternalInput")
    ref = nc.dram_tensor("ref", (C, HW), F32, kind="ExternalInput")
    mask = nc.dram_tensor("mask", (HW,), F32, kind="ExternalInput")
    wT = nc.dram_tensor("wT", (C, CQ), BF16, kind="ExternalInput")
    out = nc.dram_tensor("out", (2 * C, HW), F32, kind="ExternalOutput")
    with tile.TileContext(nc) as tc:
        _build_body(tc, src, ref, mask, wT, out)
    nc.compile()
    return nc


_CACHE = {}


def _get_nc():
    if "nc" not in _CACHE:
        _CACHE["nc"] = build()
    return _CACHE["nc"]


def _in_maps(src_mask, src_feature, ref_feature, conv_w):
    import ml_dtypes

    n_batch = src_feature.shape[0]
    wT = np.ascontiguousarray(
        np.asarray(conv_w, dtype=np.float32).T.astype(ml_dtypes.bfloat16)
    )
    maps = []
    for n in range(n_batch):
        maps.append(
            {
                "src": np.ascontiguousarray(
                    np.asarray(src_feature[n], dtype=np.float32).reshape(C, HW)
                ),
                "ref": np.ascontiguousarray(
                    np.asarray(ref_feature[n], dtype=np.float32).reshape(C, HW)
                ),
                "mask": np.ascontiguousarray(
                    np.asarray(src_mask[n], dtype=np.float32).reshape(HW)
                ),
                "wT": wT,
            }
        )
    return maps


def _install_ntff_hook():
    """The agent image's antenv lacks axon_hooks; recreate it so
    run_bass_kernel_spmd(trace=True) can capture NTFF profiles."""
    import sys
    import types

    if "antenv.axon_hooks" in sys.modules:
        return
    import antenv
    from trn_agent_boot.trn_boot import _ntff_profile_via_ctypes

    hook = _ntff_profile_via_ctypes("/opt/axon/libaxon_pjrt.so")
    mod = types.ModuleType("antenv.axon_hooks")
    mod._hook = hook
    mod.set_axon_ntff_profile_hook = lambda h: setattr(mod, "_hook", h)
    mod.get_axon_ntff_profile_hook = lambda: mod._hook
    sys.modules["antenv.axon_hooks"] = mod
    antenv.axon_hooks = mod


def run(src_mask, src_feature, ref_feature, conv_w, trace=False):
    """Run on 8 NeuronCores. Returns (output [N,2C,H,W], BassKernelResults)."""
    n_batch, c, h, w = src_feature.shape
    if trace:
        _install_ntff_hook()
    nc = _get_nc()
    maps = _in_maps(src_mask, src_feature, ref_feature, conv_w)
    res = bass_utils.run_bass_kernel_spmd(
        nc, maps, core_ids=list(range(NCORES)), trace=trace
    )
    out = np.stack([r["out"] for r in res.results], axis=0)
    return out.reshape(n_batch, 2 * c, h, w).astype(np.float32), res


def kernel(src_mask, src_feature, ref_feature, conv_w):
    out, _ = run(src_mask, src_feature, ref_feature, conv_w)
    return out


# revision 14
# speedup vs baseline: 1.4388x; 1.0040x over previous
"""Trainium2 Bass kernel for ExampleGuidedAttention (N=8, C=256, H=W=64).

Data-parallel over batch N across 8 NeuronCores; each core computes one
batch element's full guided attention.

Algorithm notes (per core):
  q = conv_w @ src_pix                      [64, 4096]   (PE, bf16)
  S^T[j,i] = sum_o q[o,j] q[o,i]            (PE, bf16; S symmetric; two
             j-blocks packed in the 128x128 array via tile_position)
  F[j,i] = exp(S^T[j,i] - 64 + 20*ln2)      (ACT; global shift keeps the
             fp32 exp in range; the 2^20 factor cancels against 1/Z)
  Per column-slice s (512 pixels) the F tiles span ALL j, so
  Z[i] = sum_j F[j,i] is computed per-slice with ones-vector matmuls on
  the PE, and each slice normalizes + blends + stores immediately --
  no full-image unnormalized buffer and no end-of-kernel fixup tail.

  Off-diagonal j-block tiles (28 of 32 per slice) are stored in fp8-e4m3
  and applied with DoubleRow matmuls (2 j-blocks per pass, 2x PE rate).
  The 4 diagonal-crossing tiles stay bf16 so the dominant near-diagonal
  attention terms keep full precision; all tiles share the 2^20 scale so
  they accumulate consistently in PSUM and the scale cancels in 1/Z.

  out = [ (1-m)*ref_att*invZ + m*ref ; src_att*invZ ]

The issue order software-pipelines slice s's scores/exp chain (ACT
bound, single PSUM group buffer) against slice s-1's apply matmuls so
the PE never stalls on the exp chain.
"""

import math

import numpy as np

import concourse.bass as bass
import concourse.mybir as mybir
import concourse.tile as tile
from concourse import bacc, bass_utils
from concourse.bass import ts
from concourse.alu_op_type import AluOpType

P = 128
C = 256          # feature channels
CQ = 64          # query channels
HW = 4096        # pixels per image
NB = HW // P     # 32 pixel blocks (contraction chunks)
SLICE = 512
NS = HW // SLICE  # 8 output column slices
NG = NB // 4      # 8 score groups of 4 j-blocks per slice
NCORES = 8
K_SCALE = 13.0    # F scaled by 2^13: off-diag fp8 overflow headroom to S=70

F32 = mybir.dt.float32
BF16 = mybir.dt.bfloat16
F8 = mybir.dt.float8e4
EXP = mybir.ActivationFunctionType.Exp
DR = mybir.MatmulPerfMode.DoubleRow


def _build_body(tc, src, ref, mask, wT, out):
    nc = tc.nc
    src_r = src.ap().rearrange("(ci p) j -> p ci j", p=P)   # [128, 2, 4096]
    ref_r = ref.ap().rearrange("(ci p) j -> p ci j", p=P)
    wT_r = wT.ap().rearrange("(ci p) o -> p ci o", p=P)     # [128, 2, 64]
    out_r = out.ap().rearrange("(cb p) j -> cb p j", p=P)   # [4, 128, 4096]

    with (
        tc.tile_pool(name="persist", bufs=1) as persist,
        tc.tile_pool(name="ps_sc", bufs=1, space="PSUM") as ps_sc,
        tc.tile_pool(name="ps_ap", bufs=1, space="PSUM") as ps_ap,
        tc.tile_pool(name="ps_z", bufs=1, space="PSUM") as ps_z,
        tc.tile_pool(name="dram", bufs=1, space="DRAM") as dram,
    ):
        refb = persist.tile([P, 2, HW], BF16)
        q2 = persist.tile([P, HW], BF16)
        pixT_bf = persist.tile([P, NB, 2 * C], BF16)   # [src 256 | ref 256]
        pixT_f8 = persist.tile([P, NB, 2 * C], F8)
        wT_sb = persist.tile([P, 2, CQ], BF16)
        mask_rep = persist.tile([P, HW], BF16)
        omask_rep = persist.tile([P, HW], BF16)        # 1 - mask
        mref = persist.tile([P, 2, HW], BF16)          # mask * ref
        exp_bias = persist.tile([P, 1], F32)
        ones8 = persist.tile([P, 2, 16], F8)  # 16B k-pair stride for dual-fp8 ldweights
        ones_bf = persist.tile([P, 2], BF16)
        warm_sb = persist.tile([P, SLICE], BF16)
        zrow = dram.tile([NS, SLICE], F32)
        nc.vector.memset(exp_bias, -64.0 + K_SCALE * math.log(2.0))
        nc.vector.memset(ones8, 1.0)
        nc.vector.memset(ones_bf, 1.0)
        nc.vector.memset(warm_sb, 0.0)

        nc.sync.dma_start(out=wT_sb, in_=wT_r)
        for s in range(NS):
            nc.gpsimd.dma_start(
                out=mask_rep[:, ts(s, SLICE)],
                in_=mask.ap()[ts(s, SLICE)].partition_broadcast(P),
            )

        with tc.tile_pool(name="early", bufs=4) as early, \
             tc.tile_pool(name="early1", bufs=1) as early1, \
             tc.tile_pool(name="ps_cv", bufs=1, space="PSUM") as ps_cv:
            # PE warmup: back-to-back matmuls on zeroed data keep the HAM
            # clock gate at 8/8 (2.4 GHz) while input DMAs stream in.
            warm_ps = ps_sc.tile([P, 4, SLICE], F32, name="pss", tag="pss")
            for r in range(14):
                nc.tensor.matmul(
                    warm_ps[:, r % 4, :], warm_sb[:, 0:P], warm_sb,
                    start=True, stop=True,
                )
            # warm the ACT exp table so slice 0 doesn't pay the table load
            nc.scalar.activation(
                out=warm_sb[:, 0:1], in_=warm_sb[:, 0:1], func=EXP,
                bias=exp_bias,
            )

            srcb = early1.tile([P, 2, HW], BF16)
            # fp32 inputs via fast hardware DMA.  src casts on DVE (conv ->
            # q gates everything); ref casts on ACT so the DVE can proceed
            # to q2 / fp8 work without head-of-line blocking.
            src_stages = []
            for s in range(NS):
                st = early.tile([P, 2, SLICE], F32, name="stage", tag="st")
                (nc.sync if s % 2 == 0 else nc.scalar).dma_start(
                    out=st, in_=src_r[:, :, ts(s, SLICE)]
                )
                src_stages.append(st)
            ref_stages = []
            for s in range(NS):
                st = early.tile([P, 2, SLICE], F32, name="rstage", tag="rst")
                nc.scalar.dma_start(out=st, in_=ref_r[:, :, ts(s, SLICE)])
                ref_stages.append(st)
            for s in range(NS):
                nc.vector.tensor_copy(
                    out=srcb[:, :, ts(s, SLICE)], in_=src_stages[s]
                )
            # 1x1 conv: q = wT.T @ src_pix; q into both partition halves.
            # 3 warm matmuls between slices keep the PE (and HAM clock)
            # busy while the next slice's DMA+cast lands.
            for s in range(NS):
                sl = ts(s, SLICE)
                psq = ps_cv.tile([CQ, SLICE], F32, name="psq", tag="psq")
                for ci in range(2):
                    nc.tensor.matmul(
                        psq,
                        wT_sb[:, ci, :],
                        srcb[:, ci, sl],
                        start=(ci == 0),
                        stop=(ci == 1),
                    )
                for r in range(3):
                    nc.tensor.matmul(
                        warm_ps[:, r, :], warm_sb[:, 0:P], warm_sb,
                        start=True, stop=True,
                    )
                nc.vector.tensor_copy(out=q2[0:CQ, sl], in_=psq)
                nc.vector.tensor_copy(out=q2[CQ:P, sl], in_=psq)
            # ref casts on gpsimd: keeps both the DVE (q2/fp8 casts) and the
            # ACT (exp chain) free of head-of-line blocking.
            for s in range(NS):
                nc.gpsimd.tensor_copy(
                    out=refb[:, :, ts(s, SLICE)], in_=ref_stages[s]
                )

            # XBAR transposes: pixT[p, b, c] = pix[c, b*128+p]; j-halves so
            # each starts as soon as half the casts have landed.
            for ci in range(2):
                cs = slice(ci * P, (ci + 1) * P)
                cs_r = slice(C + ci * P, C + (ci + 1) * P)
                for h in range(2):
                    jh = slice(h * (HW // 2), (h + 1) * (HW // 2))
                    bh = slice(h * (NB // 2), (h + 1) * (NB // 2))
                    nc.sync.dma_start_transpose(
                        out=pixT_bf[:, bh, cs], in_=srcb[:, ci, jh]
                    )
                    nc.sync.dma_start_transpose(
                        out=pixT_bf[:, bh, cs_r], in_=refb[:, ci, jh]
                    )
            # fp8 copy of the pixel transpose for DoubleRow matmuls; src
            # column half first, j-ascending, so apply wave A can start
            # before the ref transposes have even landed.
            for b in range(0, NB, 4):
                nc.vector.tensor_copy(
                    out=pixT_f8[:, b : b + 4, 0:C],
                    in_=pixT_bf[:, b : b + 4, 0:C],
                )
            for b in range(0, NB, 4):
                nc.gpsimd.tensor_copy(
                    out=pixT_f8[:, b : b + 4, C : 2 * C],
                    in_=pixT_bf[:, b : b + 4, C : 2 * C],
                )
            # blend precomputes: 1-m and m*ref
            nc.vector.tensor_scalar(
                out=omask_rep, in0=mask_rep, scalar1=-1.0, scalar2=1.0,
                op0=AluOpType.mult, op1=AluOpType.add,
            )
            for ci in range(2):
                nc.vector.tensor_mul(mref[:, ci, :], mask_rep, refb[:, ci, :])

        def f8slot(s, jb):
            return jb if jb < 4 * s else jb - 4

        with tc.tile_pool(name="fbuf", bufs=2) as fbuf, \
             tc.tile_pool(name="obuf", bufs=2) as obuf, \
             tc.tile_pool(name="zbuf", bufs=2) as zbuf:

            def emit_scores_group(s, g, f8, fbf):
                sl = ts(s, SLICE)
                pss = ps_sc.tile([P, 4, SLICE], F32, name="pss", tag="pss")
                for jp in range(2):
                    jb0, jb1 = 4 * g + 2 * jp, 4 * g + 2 * jp + 1
                    nc.tensor.matmul(
                        pss[:, 2 * jp, :], q2[0:CQ, ts(jb0, P)], q2[0:CQ, sl],
                        start=True, stop=True, tile_position=(0, 0),
                    )
                    nc.tensor.matmul(
                        pss[:, 2 * jp + 1, :], q2[CQ:P, ts(jb1, P)],
                        q2[CQ:P, sl],
                        start=True, stop=True, tile_position=(CQ, 0),
                    )
                if g == s:
                    nc.scalar.activation(
                        out=fbf, in_=pss, func=EXP, bias=exp_bias
                    )
                else:
                    fs = f8slot(s, 4 * g)
                    nc.scalar.activation(
                        out=f8[:, fs : fs + 4, :], in_=pss, func=EXP,
                        bias=exp_bias,
                    )

            def emit_z(s, f8, fbf):
                zps = ps_z.tile([2, SLICE], F32, name="psz", tag="psz")
                n_z = 0
                for g in range(NG):
                    if g == s:
                        for r in range(4):
                            nc.tensor.matmul(
                                zps, ones_bf, fbf[:, r, :],
                                start=(n_z == 0), stop=(n_z == 17),
                            )
                            n_z += 1
                    else:
                        fs = f8slot(s, 4 * g)
                        for r in range(2):
                            nc.tensor.matmul(
                                zps, ones8[:, :, 0:2],
                                f8[:, fs + 2 * r : fs + 2 * r + 2, :],
                                start=(n_z == 0), stop=(n_z == 17),
                                perf_mode=DR,
                            )
                            n_z += 1
                zinv_row = zbuf.tile([1, SLICE], F32, name="zr", tag="zr")
                invz_rep = zbuf.tile([P, SLICE], F32, name="zrep", tag="zrep")
                nc.vector.reciprocal(out=zinv_row, in_=zps[0:1, :])
                nc.sync.dma_start(out=zrow[s], in_=zinv_row)
                nc.sync.dma_start(
                    out=invz_rep, in_=zrow[s].partition_broadcast(P)
                )
                return invz_rep

            def emit_warm_chunk():
                wz = ps_z.tile([2, SLICE], F32, name="psz", tag="psz")
                for r in range(9):
                    nc.tensor.matmul(
                        wz, warm_sb[:, 0:2], warm_sb,
                        start=(r == 0), stop=(r == 8),
                    )

            def build_mm_list(s):
                """Ordered apply matmuls: wave-major, then half, then
                j-groups; start/stop flags per (wave, half) psum region."""
                lst = []
                for wave in range(2):
                    for half in range(2):
                        blk = []
                        for g in range(NG):
                            if g == s:
                                for r in range(4):
                                    blk.append([wave, half, g, r, False, 0, 0])
                            else:
                                for r in range(2):
                                    blk.append([wave, half, g, r, True, 0, 0])
                        blk[0][5] = 1
                        blk[-1][6] = 1
                        lst.extend(blk)
                return lst

            def emit_apply_mm(ctx, e):
                wave, half, g, r, is_dr, st, sp = e
                s, f8, fbf, pso = ctx["s"], ctx["f8"], ctx["fbf"], ctx["pso"]
                cs = slice((2 * wave + half) * P, (2 * wave + half + 1) * P)
                if is_dr:
                    fs = f8slot(s, 4 * g)
                    jb = 4 * g + 2 * r
                    nc.tensor.matmul(
                        pso[:, half, :], pixT_f8[:, jb : jb + 2, cs],
                        f8[:, fs + 2 * r : fs + 2 * r + 2, :],
                        start=bool(st), stop=bool(sp), perf_mode=DR,
                    )
                else:
                    jb = 4 * g + r
                    nc.tensor.matmul(
                        pso[:, half, :], pixT_bf[:, jb, cs], fbf[:, r, :],
                        start=bool(st), stop=bool(sp),
                    )

            def emit_finalize_wave(ctx, wave):
                s, invz_rep = ctx["s"], ctx["invz"]
                sl = ts(s, SLICE)
                outb = ctx["outb"]
                scr = obuf.tile([P, 2, SLICE], F32, name="scr", tag="scr")
                nc.vector.tensor_copy(out=scr, in_=ctx["pso"])
                if wave == 0:
                    # src_att * invz -> out rows 256..512
                    for half in range(2):
                        nc.vector.tensor_mul(
                            outb[:, 2 + half, :], scr[:, half, :], invz_rep
                        )
                else:
                    # flow = ref_att*invz*(1-m) + m*ref -> out rows 0..256
                    a_s = obuf.tile([P, SLICE], F32, name="a_s", tag="a_s")
                    nc.vector.tensor_mul(a_s, omask_rep[:, sl], invz_rep)
                    for half in range(2):
                        nc.vector.tensor_mul(
                            outb[:, half, :], scr[:, half, :], a_s
                        )
                        nc.vector.tensor_add(
                            outb[:, half, :], outb[:, half, :],
                            mref[:, half, sl],
                        )
                    oq = [nc.sync, nc.gpsimd]
                    for k in range(4):
                        oq[(s + k) % 2].dma_start(
                            out=out_r[k, :, sl], in_=outb[:, k, :]
                        )

            prev = None
            for s in range(NS):
                f8 = fbuf.tile([P, NB - 4, SLICE], F8, name="f8", tag="f8")
                fbf = fbuf.tile([P, 4, SLICE], BF16, name="fbf", tag="fbf")
                ctx = {"s": s, "f8": f8, "fbf": fbf}
                # interleave this slice's scores/exp chain with the previous
                # slice's apply matmuls in chunks of 9 per score group, so
                # the PE always has work while the exp chain serializes on
                # its single PSUM group buffer.
                for g in range(NG):
                    emit_scores_group(s, g, f8, fbf)
                    if prev is not None:
                        if g == 0:
                            prev["outb"] = obuf.tile(
                                [P, 4, SLICE], F32, name="outb", tag="outb"
                            )
                            prev["pso"] = ps_ap.tile(
                                [P, 2, SLICE], F32, name="psa", tag="psa"
                            )
                        if g == 4:
                            prev["pso"] = ps_ap.tile(
                                [P, 2, SLICE], F32, name="psa", tag="psa"
                            )
                        for e in prev["mm"][9 * g : 9 * g + 9]:
                            emit_apply_mm(prev, e)
                        if g == 3:
                            emit_finalize_wave(prev, 0)
                        elif g == 7:
                            emit_finalize_wave(prev, 1)
                    elif g < 7:
                        emit_warm_chunk()
                ctx["invz"] = emit_z(s, f8, fbf)
                ctx["mm"] = build_mm_list(s)
                prev = ctx
            # drain: last slice's apply + finalize
            prev["outb"] = obuf.tile([P, 4, SLICE], F32, name="outb", tag="outb")
            prev["pso"] = ps_ap.tile([P, 2, SLICE], F32, name="psa", tag="psa")
            for e in prev["mm"][0:36]:
                emit_apply_mm(prev, e)
            emit_finalize_wave(prev, 0)
            prev["pso"] = ps_ap.tile([P, 2, SLICE], F32, name="psa", tag="psa")
            for e in prev["mm"][36:72]:
                emit_apply_mm(prev, e)
            emit_finalize_wave(prev, 1)


def build():
    nc = bacc.Bacc(
        "TRN2",
        target_bir_lowering=False,
        debug=False,
        enable_asserts=False,
        num_devices=NCORES,
    )
    src = nc.dram_tensor("src", (C, HW), F32, kind="ExternalInput")
    ref = nc.dram_tensor("ref", (C, HW), F32, kind="ExternalInput")
    mask = nc.dram_tensor("mask", (HW,), F32, kind="ExternalInput")
    wT = nc.dram_tensor("wT", (C, CQ), BF16, kind="ExternalInput")
    out = nc.dram_tensor("out", (2 * C, HW), F32, kind="ExternalOutput")
    with tile.TileContext(nc) as tc:
        _build_body(tc, src, ref, mask, wT, out)
    nc.compile()
    return nc


_CACHE = {}


def _get_nc():
    if "nc" not in _CACHE:
        _CACHE["nc"] = build()
    return _CACHE["nc"]


def _in_maps(src_mask, src_feature, ref_feature, conv_w):
    import ml_dtypes

    n_batch = src_feature.shape[0]
    wT = np.ascontiguousarray(
        np.asarray(conv_w, dtype=np.float32).T.astype(ml_dtypes.bfloat16)
    )
    maps = []
    for n in range(n_batch):
        maps.append(
            {
                "src": np.ascontiguousarray(
                    np.asarray(src_feature[n], dtype=np.float32).reshape(C, HW)
                ),
                "ref": np.ascontiguousarray(
                    np.asarray(ref_feature[n], dtype=np.float32).reshape(C, HW)
                ),
                "mask": np.ascontiguousarray(
                    np.asarray(src_mask[n], dtype=np.float32).reshape(HW)
                ),
                "wT": wT,
            }
        )
    return maps


def _install_ntff_hook():
    """The agent image's antenv lacks axon_hooks; recreate it so
    run_bass_kernel_spmd(trace=True) can capture NTFF profiles."""
    import sys
    import types

    if "antenv.axon_hooks" in sys.modules:
        return
    import antenv
    from trn_agent_boot.trn_boot import _ntff_profile_via_ctypes

    hook = _ntff_profile_via_ctypes("/opt/axon/libaxon_pjrt.so")
    mod = types.ModuleType("antenv.axon_hooks")
    mod._hook = hook
    mod.set_axon_ntff_profile_hook = lambda h: setattr(mod, "_hook", h)
    mod.get_axon_ntff_profile_hook = lambda: mod._hook
    sys.modules["antenv.axon_hooks"] = mod
    antenv.axon_hooks = mod


def run(src_mask, src_feature, ref_feature, conv_w, trace=False):
    """Run on 8 NeuronCores. Returns (output [N,2C,H,W], BassKernelResults)."""
    n_batch, c, h, w = src_feature.shape
    if trace:
        _install_ntff_hook()
    nc = _get_nc()
    maps = _in_maps(src_mask, src_feature, ref_feature, conv_w)
    res = bass_utils.run_bass_kernel_spmd(
        nc, maps, core_ids=list(range(NCORES)), trace=trace
    )
    out = np.stack([r["out"] for r in res.results], axis=0)
    return out.reshape(n_batch, 2 * c, h, w).astype(np.float32), res


def kernel(src_mask, src_feature, ref_feature, conv_w):
    out, _ = run(src_mask, src_feature, ref_feature, conv_w)
    return out


# revision 16
# speedup vs baseline: 1.5569x; 1.0821x over previous
"""Trainium2 Bass kernel for ExampleGuidedAttention (N=8, C=256, H=W=64).

Data-parallel over batch N across 8 NeuronCores; each core computes one
batch element's full guided attention.

Algorithm notes (per core):
  q = conv_w @ src_pix                      [64, 4096]   (PE, bf16)
  S^T[j,i] = sum_o q[o,j] q[o,i]            (PE, bf16; S symmetric; two
             j-blocks packed in the 128x128 array via tile_position)
  F[j,i] = exp(S^T[j,i] - 64 + 20*ln2)      (ACT; global shift keeps the
             fp32 exp in range; the 2^20 factor cancels against 1/Z)
  Per column-slice s (512 pixels) the F tiles span ALL j, so
  Z[i] = sum_j F[j,i] is computed per-slice with ones-vector matmuls on
  the PE, and each slice normalizes + blends + stores immediately --
  no full-image unnormalized buffer and no end-of-kernel fixup tail.

  Off-diagonal j-block tiles (28 of 32 per slice) are stored in fp8-e4m3
  and applied with DoubleRow matmuls (2 j-blocks per pass, 2x PE rate).
  The 4 diagonal-crossing tiles stay bf16 so the dominant near-diagonal
  attention terms keep full precision; all tiles share the 2^20 scale so
  they accumulate consistently in PSUM and the scale cancels in 1/Z.

  out = [ (1-m)*ref_att*invZ + m*ref ; src_att*invZ ]

The issue order software-pipelines slice s's scores/exp chain (ACT
bound, single PSUM group buffer) against slice s-1's apply matmuls so
the PE never stalls on the exp chain.
"""

import math

import numpy as np

import concourse.bass as bass
import concourse.mybir as mybir
import concourse.tile as tile
from concourse import bacc, bass_utils
from concourse.bass import ts
from concourse.alu_op_type import AluOpType

P = 128
C = 256          # feature channels
CQ = 64          # query channels
HW = 4096        # pixels per image
NB = HW // P     # 32 pixel blocks (contraction chunks)
SLICE = 512
NS = HW // SLICE  # 8 output column slices
NG = NB // 4      # 8 score groups of 4 j-blocks per slice
NCORES = 8
K_SCALE = 13.0    # F scaled by 2^13: off-diag fp8 overflow headroom to S=70

F32 = mybir.dt.float32
BF16 = mybir.dt.bfloat16
F8 = mybir.dt.float8e4
EXP = mybir.ActivationFunctionType.Exp
DR = mybir.MatmulPerfMode.DoubleRow


def _build_body(tc, src, ref, mask, wT, out):
    nc = tc.nc
    src_r = src.ap().rearrange("(ci p) j -> p ci j", p=P)   # [128, 2, 4096]
    ref_r = ref.ap().rearrange("(ci p) j -> p ci j", p=P)
    wT_r = wT.ap().rearrange("(ci p) o -> p ci o", p=P)     # [128, 2, 64]
    out_r = out.ap().rearrange("(cb p) j -> cb p j", p=P)   # [4, 128, 4096]

    with (
        tc.tile_pool(name="persist", bufs=1) as persist,
        tc.tile_pool(name="ps_sc", bufs=1, space="PSUM") as ps_sc,
        tc.tile_pool(name="ps_ap", bufs=1, space="PSUM") as ps_ap,
        tc.tile_pool(name="ps_z", bufs=2, space="PSUM") as ps_z,
        tc.tile_pool(name="dram", bufs=1, space="DRAM") as dram,
    ):
        refb = persist.tile([P, 2, HW], BF16)
        q2 = persist.tile([P, HW], BF16)
        pixT_bf = persist.tile([P, NB, 2 * C], BF16)   # [src 256 | ref 256]
        pixT_f8 = persist.tile([P, NB, 2 * C], F8)
        wT_sb = persist.tile([P, 2, CQ], BF16)
        mask_rep = persist.tile([P, HW], BF16)
        omask_rep = persist.tile([P, HW], BF16)        # 1 - mask
        mref = persist.tile([P, 2, HW], BF16)          # mask * ref
        exp_bias = persist.tile([P, 1], F32)
        ones8 = persist.tile([P, 2, 16], F8)  # 16B k-pair stride for dual-fp8 ldweights
        ones_bf = persist.tile([P, 2], BF16)
        warm_sb = persist.tile([P, SLICE], BF16)
        zrow = dram.tile([NS, SLICE], F32)
        nc.vector.memset(exp_bias, -64.0 + K_SCALE * math.log(2.0))
        nc.vector.memset(ones8, 1.0)
        nc.vector.memset(ones_bf, 1.0)
        nc.vector.memset(warm_sb, 0.0)

        nc.sync.dma_start(out=wT_sb, in_=wT_r)
        for s in range(NS):
            nc.gpsimd.dma_start(
                out=mask_rep[:, ts(s, SLICE)],
                in_=mask.ap()[ts(s, SLICE)].partition_broadcast(P),
            )

        with tc.tile_pool(name="early", bufs=4) as early, \
             tc.tile_pool(name="early1", bufs=1) as early1:
            # PE warmup: back-to-back matmuls on zeroed data keep the HAM
            # clock gate at 8/8 (2.4 GHz) while input DMAs stream in.
            warm_ps = ps_sc.tile([P, 4, SLICE], F32, name="pss", tag="pss")
            for r in range(20):
                nc.tensor.matmul(
                    warm_ps[:, r % 4, :], warm_sb[:, 0:P], warm_sb,
                    start=True, stop=True,
                )
            # warm the ACT exp table so slice 0 doesn't pay the table load
            nc.scalar.activation(
                out=warm_sb[:, 0:1], in_=warm_sb[:, 0:1], func=EXP,
                bias=exp_bias,
            )

            srcb = early1.tile([P, 2, HW], BF16)
            # fp32 inputs via fast hardware DMA.  src casts on DVE (conv ->
            # q gates everything); ref casts on ACT so the DVE can proceed
            # to q2 / fp8 work without head-of-line blocking.
            CH = 1024  # input DMA chunk: 4KB contiguous rows for HW DGE
            src_stages = []
            for c in range(4):
                st = early.tile([P, 2, CH], F32, name="stage", tag="st")
                (nc.sync if c % 2 == 0 else nc.scalar).dma_start(
                    out=st, in_=src_r[:, :, ts(c, CH)]
                )
                src_stages.append(st)
            ref_stages = []
            for c in range(4):
                st = early.tile([P, 2, CH], F32, name="rstage", tag="rst")
                (nc.sync if c % 2 == 0 else nc.scalar).dma_start(
                    out=st, in_=ref_r[:, :, ts(c, CH)]
                )
                ref_stages.append(st)
            for c in range(4):
                nc.vector.tensor_copy(
                    out=srcb[:, :, ts(c, CH)], in_=src_stages[c]
                )
            # 1x1 conv: q = wT.T @ src_pix; q into both partition halves.
            # 3 warm matmuls between slices keep the PE (and HAM clock)
            # busy while the next slice's DMA+cast lands.
            for s in range(NS):
                sl = ts(s, SLICE)
                psq = ps_z.tile([CQ, SLICE], F32, name="psz", tag="psz")
                for ci in range(2):
                    nc.tensor.matmul(
                        psq,
                        wT_sb[:, ci, :],
                        srcb[:, ci, sl],
                        start=(ci == 0),
                        stop=(ci == 1),
                    )
                for r in range(3):
                    nc.tensor.matmul(
                        warm_ps[:, r, :], warm_sb[:, 0:P], warm_sb,
                        start=True, stop=True,
                    )
                nc.vector.tensor_copy(out=q2[0:CQ, sl], in_=psq)
                nc.vector.tensor_copy(out=q2[CQ:P, sl], in_=psq)
            # ref casts on gpsimd: keeps both the DVE (q2/fp8 casts) and the
            # ACT (exp chain) free of head-of-line blocking.
            for c in range(4):
                nc.gpsimd.tensor_copy(
                    out=refb[:, :, ts(c, CH)], in_=ref_stages[c]
                )

            # XBAR transposes: pixT[p, b, c] = pix[c, b*128+p]; j-halves so
            # each starts as soon as half the casts have landed.
            for ci in range(2):
                cs = slice(ci * P, (ci + 1) * P)
                cs_r = slice(C + ci * P, C + (ci + 1) * P)
                for h in range(2):
                    jh = slice(h * (HW // 2), (h + 1) * (HW // 2))
                    bh = slice(h * (NB // 2), (h + 1) * (NB // 2))
                    nc.sync.dma_start_transpose(
                        out=pixT_bf[:, bh, cs], in_=srcb[:, ci, jh]
                    )
                    nc.sync.dma_start_transpose(
                        out=pixT_bf[:, bh, cs_r], in_=refb[:, ci, jh]
                    )
            # fp8 copy of the pixel transpose for DoubleRow matmuls; src
            # column half first, j-ascending, so apply wave A can start
            # before the ref transposes have even landed.
            for b in range(0, NB, 4):
                nc.vector.tensor_copy(
                    out=pixT_f8[:, b : b + 4, 0:C],
                    in_=pixT_bf[:, b : b + 4, 0:C],
                )
            for b in range(0, NB, 4):
                nc.gpsimd.tensor_copy(
                    out=pixT_f8[:, b : b + 4, C : 2 * C],
                    in_=pixT_bf[:, b : b + 4, C : 2 * C],
                )
            # blend precomputes: 1-m and m*ref
            nc.vector.tensor_scalar(
                out=omask_rep, in0=mask_rep, scalar1=-1.0, scalar2=1.0,
                op0=AluOpType.mult, op1=AluOpType.add,
            )
            for ci in range(2):
                nc.vector.tensor_mul(mref[:, ci, :], mask_rep, refb[:, ci, :])

        def f8slot(s, jb):
            return jb if jb < 4 * s else jb - 4

        with tc.tile_pool(name="fbuf", bufs=2) as fbuf, \
             tc.tile_pool(name="obuf", bufs=3) as obuf, \
             tc.tile_pool(name="zbuf", bufs=2) as zbuf:

            def emit_scores_group(s, g, f8, fbf):
                sl = ts(s, SLICE)
                pss = ps_sc.tile([P, 4, SLICE], F32, name="pss", tag="pss")
                for jp in range(2):
                    jb0, jb1 = 4 * g + 2 * jp, 4 * g + 2 * jp + 1
                    nc.tensor.matmul(
                        pss[:, 2 * jp, :], q2[0:CQ, ts(jb0, P)], q2[0:CQ, sl],
                        start=True, stop=True, tile_position=(0, 0),
                    )
                    nc.tensor.matmul(
                        pss[:, 2 * jp + 1, :], q2[CQ:P, ts(jb1, P)],
                        q2[CQ:P, sl],
                        start=True, stop=True, tile_position=(CQ, 0),
                    )
                if g == s:
                    nc.scalar.activation(
                        out=fbf, in_=pss, func=EXP, bias=exp_bias
                    )
                else:
                    fs = f8slot(s, 4 * g)
                    nc.scalar.activation(
                        out=f8[:, fs : fs + 4, :], in_=pss, func=EXP,
                        bias=exp_bias,
                    )

            def emit_z(s, f8, fbf):
                zps = ps_z.tile([2, SLICE], F32, name="psz", tag="psz")
                n_z = 0
                for g in range(NG):
                    if g == s:
                        for r in range(4):
                            nc.tensor.matmul(
                                zps, ones_bf, fbf[:, r, :],
                                start=(n_z == 0), stop=(n_z == 17),
                            )
                            n_z += 1
                    else:
                        fs = f8slot(s, 4 * g)
                        for r in range(2):
                            nc.tensor.matmul(
                                zps, ones8[:, :, 0:2],
                                f8[:, fs + 2 * r : fs + 2 * r + 2, :],
                                start=(n_z == 0), stop=(n_z == 17),
                                perf_mode=DR,
                            )
                            n_z += 1
                zinv_row = zbuf.tile([1, SLICE], F32, name="zr", tag="zr")
                invz_rep = zbuf.tile([P, SLICE], F32, name="zrep", tag="zrep")
                nc.vector.reciprocal_approx_fast(out=zinv_row, in_=zps[0:1, :])
                nc.sync.dma_start(out=zrow[s], in_=zinv_row)
                nc.sync.dma_start(
                    out=invz_rep, in_=zrow[s].partition_broadcast(P)
                )
                return invz_rep

            def emit_warm_chunk():
                wz = ps_z.tile([2, SLICE], F32, name="psz", tag="psz")
                for r in range(9):
                    nc.tensor.matmul(
                        wz, warm_sb[:, 0:2], warm_sb,
                        start=(r == 0), stop=(r == 8),
                    )

            def build_mm_list(s):
                """Ordered apply matmuls: wave-major, then half, then
                j-groups; start/stop flags per (wave, half) psum region."""
                lst = []
                for wave in range(2):
                    for half in range(2):
                        blk = []
                        for g in range(NG):
                            if g == s:
                                for r in range(4):
                                    blk.append([wave, half, g, r, False, 0, 0])
                            else:
                                for r in range(2):
                                    blk.append([wave, half, g, r, True, 0, 0])
                        blk[0][5] = 1
                        blk[-1][6] = 1
                        lst.extend(blk)
                return lst

            def emit_apply_mm(ctx, e):
                wave, half, g, r, is_dr, st, sp = e
                s, f8, fbf, pso = ctx["s"], ctx["f8"], ctx["fbf"], ctx["pso"]
                cs = slice((2 * wave + half) * P, (2 * wave + half + 1) * P)
                if is_dr:
                    fs = f8slot(s, 4 * g)
                    jb = 4 * g + 2 * r
                    nc.tensor.matmul(
                        pso[:, half, :], pixT_f8[:, jb : jb + 2, cs],
                        f8[:, fs + 2 * r : fs + 2 * r + 2, :],
                        start=bool(st), stop=bool(sp), perf_mode=DR,
                    )
                else:
                    jb = 4 * g + r
                    nc.tensor.matmul(
                        pso[:, half, :], pixT_bf[:, jb, cs], fbf[:, r, :],
                        start=bool(st), stop=bool(sp),
                    )

            def emit_finalize_wave(ctx, wave):
                s, invz_rep = ctx["s"], ctx["invz"]
                sl = ts(s, SLICE)
                outb = ctx["outb"]
                scr = obuf.tile([P, 2, SLICE], F32, name="scr", tag="scr")
                nc.vector.tensor_copy(out=scr, in_=ctx["pso"])
                if wave == 0:
                    # src_att * invz -> out rows 256..512
                    for half in range(2):
                        nc.vector.tensor_mul(
                            outb[:, 2 + half, :], scr[:, half, :], invz_rep
                        )
                else:
                    # flow = ref_att*invz*(1-m) + m*ref -> out rows 0..256
                    a_s = obuf.tile([P, SLICE], F32, name="a_s", tag="a_s")
                    nc.vector.tensor_mul(a_s, omask_rep[:, sl], invz_rep)
                    for half in range(2):
                        nc.vector.tensor_mul(
                            outb[:, half, :], scr[:, half, :], a_s
                        )
                        nc.vector.tensor_add(
                            outb[:, half, :], outb[:, half, :],
                            mref[:, half, sl],
                        )
                    oq = [nc.sync, nc.gpsimd]
                    for k in range(4):
                        oq[(s + k) % 2].dma_start(
                            out=out_r[k, :, sl], in_=outb[:, k, :]
                        )

            prev = None
            for s in range(NS):
                f8 = fbuf.tile([P, NB - 4, SLICE], F8, name="f8", tag="f8")
                fbf = fbuf.tile([P, 4, SLICE], BF16, name="fbf", tag="fbf")
                ctx = {"s": s, "f8": f8, "fbf": fbf}
                # interleave this slice's scores/exp chain with the previous
                # slice's apply matmuls in chunks of 9 per score group, so
                # the PE always has work while the exp chain serializes on
                # its single PSUM group buffer.
                for g in range(NG):
                    emit_scores_group(s, g, f8, fbf)
                    if prev is not None:
                        if g == 0:
                            prev["outb"] = obuf.tile(
                                [P, 4, SLICE], F32, name="outb", tag="outb"
                            )
                            prev["pso"] = ps_ap.tile(
                                [P, 2, SLICE], F32, name="psa", tag="psa"
                            )
                        if g == 4:
                            prev["pso"] = ps_ap.tile(
                                [P, 2, SLICE], F32, name="psa", tag="psa"
                            )
                        for e in prev["mm"][9 * g : 9 * g + 9]:
                            emit_apply_mm(prev, e)
                        if g == 3:
                            emit_finalize_wave(prev, 0)
                        elif g == 7:
                            emit_finalize_wave(prev, 1)
                    elif g < 7:
                        emit_warm_chunk()
                ctx["invz"] = emit_z(s, f8, fbf)
                ctx["mm"] = build_mm_list(s)
                prev = ctx
            # drain: last slice's apply + finalize
            prev["outb"] = obuf.tile([P, 4, SLICE], F32, name="outb", tag="outb")
            prev["pso"] = ps_ap.tile([P, 2, SLICE], F32, name="psa", tag="psa")
            for e in prev["mm"][0:36]:
                emit_apply_mm(prev, e)
            emit_finalize_wave(prev, 0)
            prev["pso"] = ps_ap.tile([P, 2, SLICE], F32, name="psa", tag="psa")
            for e in prev["mm"][36:72]:
                emit_apply_mm(prev, e)
            emit_finalize_wave(prev, 1)


def build():
    nc = bacc.Bacc(
        "TRN2",
        target_bir_lowering=False,
        debug=False,
        enable_asserts=False,
        num_devices=NCORES,
    )
    src = nc.dram_tensor("src", (C, HW), F32, kind="ExternalInput")
    ref = nc.dram_tensor("ref", (C, HW), F32, kind="ExternalInput")
    mask = nc.dram_tensor("mask", (HW,), F32, kind="ExternalInput")
    wT = nc.dram_tensor("wT", (C, CQ), BF16, kind="ExternalInput")
    out = nc.dram_tensor("out", (2 * C, HW), F32, kind="ExternalOutput")
    with tile.TileContext(nc) as tc:
        _build_body(tc, src, ref, mask, wT, out)
    nc.compile()
    return nc


_CACHE = {}


def _get_nc():
    if "nc" not in _CACHE:
        _CACHE["nc"] = build()
    return _CACHE["nc"]


def _in_maps(src_mask, src_feature, ref_feature, conv_w):
    import ml_dtypes

    n_batch = src_feature.shape[0]
    wT = np.ascontiguousarray(
        np.asarray(conv_w, dtype=np.float32).T.astype(ml_dtypes.bfloat16)
    )
    maps = []
    for n in range(n_batch):
        maps.append(
            {
                "src": np.ascontiguousarray(
                    np.asarray(src_feature[n], dtype=np.float32).reshape(C, HW)
                ),
                "ref": np.ascontiguousarray(
                    np.asarray(ref_feature[n], dtype=np.float32).reshape(C, HW)
                ),
                "mask": np.ascontiguousarray(
                    np.asarray(src_mask[n], dtype=np.float32).reshape(HW)
                ),
                "wT": wT,
            }
        )
    return maps


def _install_ntff_hook():
    """The agent image's antenv lacks axon_hooks; recreate it so
    run_bass_kernel_spmd(trace=True) can capture NTFF profiles."""
    import sys
    import types

    if "antenv.axon_hooks" in sys.modules:
        return
    import antenv
    from trn_agent_boot.trn_boot import _ntff_profile_via_ctypes

    hook = _ntff_profile_via_ctypes("/opt/axon/libaxon_pjrt.so")
    mod = types.ModuleType("antenv.axon_hooks")
    mod._hook = hook
    mod.set_axon_ntff_profile_hook = lambda h: setattr(mod, "_hook", h)
    mod.get_axon_ntff_profile_hook = lambda: mod._hook
    sys.modules["antenv.axon_hooks"] = mod
    antenv.axon_hooks = mod


def run(src_mask, src_feature, ref_feature, conv_w, trace=False):
    """Run on 8 NeuronCores. Returns (output [N,2C,H,W], BassKernelResults)."""
    n_batch, c, h, w = src_feature.shape
    if trace:
        _install_ntff_hook()
    nc = _get_nc()
    maps = _in_maps(src_mask, src_feature, ref_feature, conv_w)
    res = bass_utils.run_bass_kernel_spmd(
        nc, maps, core_ids=list(range(NCORES)), trace=trace
    )
    out = np.stack([r["out"] for r in res.results], axis=0)
    return out.reshape(n_batch, 2 * c, h, w).astype(np.float32), res


def kernel(src_mask, src_feature, ref_feature, conv_w):
    out, _ = run(src_mask, src_feature, ref_feature, conv_w)
    return out


# revision 19
# speedup vs baseline: 1.6042x; 1.0304x over previous
"""Trainium2 Bass kernel for ExampleGuidedAttention (N=8, C=256, H=W=64).

Data-parallel over batch N across 8 NeuronCores; each core computes one
batch element's full guided attention.

Algorithm notes (per core):
  q = conv_w @ src_pix                      [64, 4096]   (PE, bf16)
  S^T[j,i] = sum_o q[o,j] q[o,i]            (PE, bf16; S symmetric; two
             j-blocks packed in the 128x128 array via tile_position)
  F[j,i] = exp(S^T[j,i] - 64 + 20*ln2)      (ACT; global shift keeps the
             fp32 exp in range; the 2^20 factor cancels against 1/Z)
  Per column-slice s (512 pixels) the F tiles span ALL j, so
  Z[i] = sum_j F[j,i] is computed per-slice with ones-vector matmuls on
  the PE, and each slice normalizes + blends + stores immediately --
  no full-image unnormalized buffer and no end-of-kernel fixup tail.

  Off-diagonal j-block tiles (28 of 32 per slice) are stored in fp8-e4m3
  and applied with DoubleRow matmuls (2 j-blocks per pass, 2x PE rate).
  The 4 diagonal-crossing tiles stay bf16 so the dominant near-diagonal
  attention terms keep full precision; all tiles share the 2^20 scale so
  they accumulate consistently in PSUM and the scale cancels in 1/Z.

  out = [ (1-m)*ref_att*invZ + m*ref ; src_att*invZ ]

The issue order software-pipelines slice s's scores/exp chain (ACT
bound, single PSUM group buffer) against slice s-1's apply matmuls so
the PE never stalls on the exp chain.
"""

import math

import numpy as np

import concourse.bass as bass
import concourse.mybir as mybir
import concourse.tile as tile
from concourse import bacc, bass_utils
from concourse.bass import ts
from concourse.alu_op_type import AluOpType

P = 128
C = 256          # feature channels
CQ = 64          # query channels
HW = 4096        # pixels per image
NB = HW // P     # 32 pixel blocks (contraction chunks)
SLICE = 512
NS = HW // SLICE  # 8 output column slices
NG = NB // 4      # 8 score groups of 4 j-blocks per slice
NCORES = 8
K_SCALE = 13.0    # F scaled by 2^13: off-diag fp8 overflow headroom to S=70

F32 = mybir.dt.float32
BF16 = mybir.dt.bfloat16
F8 = mybir.dt.float8e4
EXP = mybir.ActivationFunctionType.Exp
DR = mybir.MatmulPerfMode.DoubleRow


def _build_body(tc, src, ref, mask, wT, out):
    nc = tc.nc
    src_r = src.ap().rearrange("(ci p) j -> p ci j", p=P)   # [128, 2, 4096]
    ref_r = ref.ap().rearrange("(ci p) j -> p ci j", p=P)
    wT_r = wT.ap().rearrange("(ci p) o -> p ci o", p=P)     # [128, 2, 64]
    out_r = out.ap().rearrange("(cb p) j -> cb p j", p=P)   # [4, 128, 4096]

    with (
        tc.tile_pool(name="persist", bufs=1) as persist,
        tc.tile_pool(name="ps_sc", bufs=1, space="PSUM") as ps_sc,
        tc.tile_pool(name="ps_ap", bufs=1, space="PSUM") as ps_ap,
        tc.tile_pool(name="ps_z", bufs=2, space="PSUM") as ps_z,
        tc.tile_pool(name="dram", bufs=1, space="DRAM") as dram,
    ):
        refb_h = [
            persist.tile([P, 2, HW // 2], BF16, name=f"refb{h}")
            for h in range(2)
        ]
        q2 = persist.tile([P, HW], BF16)
        pixT_bf = persist.tile([P, NB, 2 * C], BF16)   # [src 256 | ref 256]
        pixT_f8 = persist.tile([P, NB, 2 * C], F8)
        wT_sb = persist.tile([P, 2, CQ], BF16)
        mask_rep = persist.tile([P, HW], BF16)
        omask_rep = persist.tile([P, HW], BF16)        # 1 - mask
        mref = persist.tile([P, 2, HW], BF16)          # mask * ref
        exp_bias = persist.tile([P, 1], F32)
        ones8 = persist.tile([P, 2, 16], F8)  # 16B k-pair stride for dual-fp8 ldweights
        ones_bf = persist.tile([P, 2], BF16)
        warm_sb = persist.tile([P, SLICE], BF16)
        zrow = dram.tile([NS, SLICE], F32)
        nc.vector.memset(exp_bias, -64.0 + K_SCALE * math.log(2.0))
        nc.vector.memset(ones8, 1.0)
        nc.vector.memset(ones_bf, 1.0)
        nc.vector.memset(warm_sb, 0.0)

        nc.sync.dma_start(out=wT_sb, in_=wT_r)
        for s in range(NS):
            nc.gpsimd.dma_start(
                out=mask_rep[:, ts(s, SLICE)],
                in_=mask.ap()[ts(s, SLICE)].partition_broadcast(P),
            )

        with tc.tile_pool(name="early", bufs=4) as early, \
             tc.tile_pool(name="early1", bufs=1) as early1:
            # PE warmup: back-to-back matmuls on zeroed data keep the HAM
            # clock gate at 8/8 (2.4 GHz) while input DMAs stream in.
            warm_ps = ps_sc.tile([P, 4, SLICE], F32, name="pss", tag="pss")
            for r in range(20):
                nc.tensor.matmul(
                    warm_ps[:, r % 4, :], warm_sb[:, 0:P], warm_sb,
                    start=True, stop=True,
                )
            # warm the ACT exp table so slice 0 doesn't pay the table load
            nc.scalar.activation(
                out=warm_sb[:, 0:1], in_=warm_sb[:, 0:1], func=EXP,
                bias=exp_bias,
            )

            srcb_h = [
                early1.tile([P, 2, HW // 2], BF16, name=f"srcb{h}")
                for h in range(2)
            ]
            # fp32 inputs via fast hardware DMA.  src casts on DVE (conv ->
            # q gates everything); ref casts on ACT so the DVE can proceed
            # to q2 / fp8 work without head-of-line blocking.
            # src/ref in 8+8 0.5MB chunks: each dma_start lands on its own
            # hardware queue (~85GB/s each), so many small concurrent
            # transfers beat few big ones.
            src_stages, ref_stages = [], []
            for c in range(NS):
                st = early.tile([P, 2, SLICE], F32, name="stage", tag="st")
                (nc.sync if c % 2 == 0 else nc.scalar).dma_start(
                    out=st, in_=src_r[:, :, ts(c, SLICE)]
                )
                src_stages.append(st)
            for c in range(NS):
                st = early.tile([P, 2, SLICE], F32, name="rstage", tag="rst")
                (nc.sync if c % 2 == 0 else nc.scalar).dma_start(
                    out=st, in_=ref_r[:, :, ts(c, SLICE)]
                )
                ref_stages.append(st)
            for c in range(NS):
                nc.vector.tensor_copy(
                    out=srcb_h[c // 4][:, :, ts(c % 4, SLICE)],
                    in_=src_stages[c],
                )
            # 1x1 conv: q = wT.T @ src_pix; q into both partition halves.
            # 3 warm matmuls between slices keep the PE (and HAM clock)
            # busy while the next slice's DMA+cast lands.
            for s in range(NS):
                sl = ts(s, SLICE)
                psq = ps_z.tile([CQ, SLICE], F32, name="psz", tag="psz")
                for ci in range(2):
                    nc.tensor.matmul(
                        psq,
                        wT_sb[:, ci, :],
                        srcb_h[s // 4][:, ci, ts(s % 4, SLICE)],
                        start=(ci == 0),
                        stop=(ci == 1),
                    )
                for r in range(3):
                    nc.tensor.matmul(
                        warm_ps[:, r, :], warm_sb[:, 0:P], warm_sb,
                        start=True, stop=True,
                    )
                nc.vector.tensor_copy(out=q2[0:CQ, sl], in_=psq)
                nc.vector.tensor_copy(out=q2[CQ:P, sl], in_=psq)
            for c in range(NS):
                nc.vector.tensor_copy(
                    out=refb_h[c // 4][:, :, ts(c % 4, SLICE)],
                    in_=ref_stages[c],
                )
            # XBAR transposes: pixT[p, b, c] = pix[c, b*128+p].  The j-half
            # tile split lets each transpose start as soon as half the
            # casts have landed; src halves first (apply wave A needs them).
            for h in range(2):
                bh = slice(h * (NB // 2), (h + 1) * (NB // 2))
                for ci in range(2):
                    nc.sync.dma_start_transpose(
                        out=pixT_bf[:, bh, slice(ci * P, (ci + 1) * P)],
                        in_=srcb_h[h][:, ci, :],
                    )
            for h in range(2):
                bh = slice(h * (NB // 2), (h + 1) * (NB // 2))
                for ci in range(2):
                    nc.sync.dma_start_transpose(
                        out=pixT_bf[:, bh, slice(C + ci * P, C + (ci + 1) * P)],
                        in_=refb_h[h][:, ci, :],
                    )
            # fp8 copy of the pixel transpose for DoubleRow matmuls; src
            # column half first, j-ascending, so apply wave A can start
            # before the ref transposes have even landed.
            for b in range(0, NB, 4):
                nc.vector.tensor_copy(
                    out=pixT_f8[:, b : b + 4, 0:C],
                    in_=pixT_bf[:, b : b + 4, 0:C],
                )
            for b in range(0, NB, 4):
                nc.vector.tensor_copy(
                    out=pixT_f8[:, b : b + 4, C : 2 * C],
                    in_=pixT_bf[:, b : b + 4, C : 2 * C],
                )
            # blend precomputes: 1-m and m*ref
            nc.vector.tensor_scalar(
                out=omask_rep, in0=mask_rep, scalar1=-1.0, scalar2=1.0,
                op0=AluOpType.mult, op1=AluOpType.add,
            )
            for ci in range(2):
                for h in range(2):
                    jh = ts(h, HW // 2)
                    nc.vector.tensor_mul(
                        mref[:, ci, jh], mask_rep[:, jh], refb_h[h][:, ci, :]
                    )

        def f8slot(s, jb):
            return jb if jb < 4 * s else jb - 4

        with tc.tile_pool(name="fbuf", bufs=2) as fbuf, \
             tc.tile_pool(name="obuf", bufs=3) as obuf, \
             tc.tile_pool(name="zbuf", bufs=2) as zbuf:

            def emit_scores_group(s, g, f8, fbf):
                sl = ts(s, SLICE)
                pss = ps_sc.tile([P, 4, SLICE], F32, name="pss", tag="pss")
                for jp in range(2):
                    jb0, jb1 = 4 * g + 2 * jp, 4 * g + 2 * jp + 1
                    nc.tensor.matmul(
                        pss[:, 2 * jp, :], q2[0:CQ, ts(jb0, P)], q2[0:CQ, sl],
                        start=True, stop=True, tile_position=(0, 0),
                    )
                    nc.tensor.matmul(
                        pss[:, 2 * jp + 1, :], q2[CQ:P, ts(jb1, P)],
                        q2[CQ:P, sl],
                        start=True, stop=True, tile_position=(CQ, 0),
                    )
                if g == s:
                    nc.scalar.activation(
                        out=fbf, in_=pss, func=EXP, bias=exp_bias
                    )
                else:
                    fs = f8slot(s, 4 * g)
                    nc.scalar.activation(
                        out=f8[:, fs : fs + 4, :], in_=pss, func=EXP,
                        bias=exp_bias,
                    )

            def emit_z(s, f8, fbf):
                zps = ps_z.tile([2, SLICE], F32, name="psz", tag="psz")
                n_z = 0
                for g in range(NG):
                    if g == s:
                        for r in range(4):
                            nc.tensor.matmul(
                                zps, ones_bf, fbf[:, r, :],
                                start=(n_z == 0), stop=(n_z == 17),
                            )
                            n_z += 1
                    else:
                        fs = f8slot(s, 4 * g)
                        for r in range(2):
                            nc.tensor.matmul(
                                zps, ones8[:, :, 0:2],
                                f8[:, fs + 2 * r : fs + 2 * r + 2, :],
                                start=(n_z == 0), stop=(n_z == 17),
                                perf_mode=DR,
                            )
                            n_z += 1
                zinv_row = zbuf.tile([1, SLICE], F32, name="zr", tag="zr")
                invz_rep = zbuf.tile([P, SLICE], F32, name="zrep", tag="zrep")
                nc.vector.reciprocal_approx_fast(out=zinv_row, in_=zps[0:1, :])
                nc.gpsimd.dma_start(out=zrow[s], in_=zinv_row)
                nc.gpsimd.dma_start(
                    out=invz_rep, in_=zrow[s].partition_broadcast(P)
                )
                return invz_rep

            def emit_warm_chunk():
                wz = ps_z.tile([2, SLICE], F32, name="psz", tag="psz")
                for r in range(9):
                    nc.tensor.matmul(
                        wz, warm_sb[:, 0:2], warm_sb,
                        start=(r == 0), stop=(r == 8),
                    )

            def build_mm_list(s):
                """Ordered apply matmuls: wave-major, then half, then
                j-groups; start/stop flags per (wave, half) psum region."""
                lst = []
                for wave in range(2):
                    for half in range(2):
                        blk = []
                        for g in range(NG):
                            if g == s:
                                for r in range(4):
                                    blk.append([wave, half, g, r, False, 0, 0])
                            else:
                                for r in range(2):
                                    blk.append([wave, half, g, r, True, 0, 0])
                        blk[0][5] = 1
                        blk[-1][6] = 1
                        lst.extend(blk)
                return lst

            def emit_apply_mm(ctx, e):
                wave, half, g, r, is_dr, st, sp = e
                s, f8, fbf, pso = ctx["s"], ctx["f8"], ctx["fbf"], ctx["pso"]
                cs = slice((2 * wave + half) * P, (2 * wave + half + 1) * P)
                if is_dr:
                    fs = f8slot(s, 4 * g)
                    jb = 4 * g + 2 * r
                    nc.tensor.matmul(
                        pso[:, half, :], pixT_f8[:, jb : jb + 2, cs],
                        f8[:, fs + 2 * r : fs + 2 * r + 2, :],
                        start=bool(st), stop=bool(sp), perf_mode=DR,
                    )
                else:
                    jb = 4 * g + r
                    nc.tensor.matmul(
                        pso[:, half, :], pixT_bf[:, jb, cs], fbf[:, r, :],
                        start=bool(st), stop=bool(sp),
                    )

            def emit_finalize_wave(ctx, wave):
                s, invz_rep = ctx["s"], ctx["invz"]
                sl = ts(s, SLICE)
                outb = ctx["outb"]
                scr = obuf.tile([P, 2, SLICE], F32, name="scr", tag="scr")
                nc.vector.tensor_copy(out=scr, in_=ctx["pso"])
                if wave == 0:
                    # src_att * invz -> out rows 256..512
                    for half in range(2):
                        nc.vector.tensor_mul(
                            outb[:, 2 + half, :], scr[:, half, :], invz_rep
                        )
                else:
                    # flow = ref_att*invz*(1-m) + m*ref -> out rows 0..256
                    a_s = obuf.tile([P, SLICE], F32, name="a_s", tag="a_s")
                    nc.vector.tensor_mul(a_s, omask_rep[:, sl], invz_rep)
                    for half in range(2):
                        nc.vector.tensor_mul(
                            outb[:, half, :], scr[:, half, :], a_s
                        )
                        nc.vector.tensor_add(
                            outb[:, half, :], outb[:, half, :],
                            mref[:, half, sl],
                        )
                    oq = [nc.sync, nc.gpsimd]
                    for k in range(4):
                        oq[(s + k) % 2].dma_start(
                            out=out_r[k, :, sl], in_=outb[:, k, :]
                        )

            prev = None
            for s in range(NS):
                f8 = fbuf.tile([P, NB - 4, SLICE], F8, name="f8", tag="f8")
                fbf = fbuf.tile([P, 4, SLICE], BF16, name="fbf", tag="fbf")
                ctx = {"s": s, "f8": f8, "fbf": fbf}
                # interleave this slice's scores/exp chain with the previous
                # slice's apply matmuls in chunks of 9 per score group, so
                # the PE always has work while the exp chain serializes on
                # its single PSUM group buffer.
                for g in range(NG):
                    emit_scores_group(s, g, f8, fbf)
                    if prev is not None:
                        if g == 0:
                            prev["outb"] = obuf.tile(
                                [P, 4, SLICE], F32, name="outb", tag="outb"
                            )
                            prev["pso"] = ps_ap.tile(
                                [P, 2, SLICE], F32, name="psa", tag="psa"
                            )
                        if g == 4:
                            prev["pso"] = ps_ap.tile(
                                [P, 2, SLICE], F32, name="psa", tag="psa"
                            )
                        for e in prev["mm"][9 * g : 9 * g + 9]:
                            emit_apply_mm(prev, e)
                        if g == 3:
                            emit_finalize_wave(prev, 0)
                        elif g == 7:
                            emit_finalize_wave(prev, 1)
                    elif g < 7:
                        emit_warm_chunk()
                ctx["invz"] = emit_z(s, f8, fbf)
                ctx["mm"] = build_mm_list(s)
                prev = ctx
            # drain: last slice's apply + finalize
            prev["outb"] = obuf.tile([P, 4, SLICE], F32, name="outb", tag="outb")
            prev["pso"] = ps_ap.tile([P, 2, SLICE], F32, name="psa", tag="psa")
            for e in prev["mm"][0:36]:
                emit_apply_mm(prev, e)
            emit_finalize_wave(prev, 0)
            prev["pso"] = ps_ap.tile([P, 2, SLICE], F32, name="psa", tag="psa")
            for e in prev["mm"][36:72]:
                emit_apply_mm(prev, e)
            emit_finalize_wave(prev, 1)


def build():
    nc = bacc.Bacc(
        "TRN2",
        target_bir_lowering=False,
        debug=False,
        enable_asserts=False,
        num_devices=NCORES,
    )
    src = nc.dram_tensor("src", (C, HW), F32, kind="ExternalInput")
    ref = nc.dram_tensor("ref", (C, HW), F32, kind="ExternalInput")
    mask = nc.dram_tensor("mask", (HW,), F32, kind="ExternalInput")
    wT = nc.dram_tensor("wT", (C, CQ), BF16, kind="ExternalInput")
    out = nc.dram_tensor("out", (2 * C, HW), F32, kind="ExternalOutput")
    with tile.TileContext(nc) as tc:
        _build_body(tc, src, ref, mask, wT, out)
    nc.compile()
    return nc


_CACHE = {}


def _get_nc():
    if "nc" not in _CACHE:
        _CACHE["nc"] = build()
    return _CACHE["nc"]


def _in_maps(src_mask, src_feature, ref_feature, conv_w):
    import ml_dtypes

    n_batch = src_feature.shape[0]
    wT = np.ascontiguousarray(
        np.asarray(conv_w, dtype=np.float32).T.astype(ml_dtypes.bfloat16)
    )
    maps = []
    for n in range(n_batch):
        maps.append(
            {
                "src": np.ascontiguousarray(
                    np.asarray(src_feature[n], dtype=np.float32).reshape(C, HW)
                ),
                "ref": np.ascontiguousarray(
                    np.asarray(ref_feature[n], dtype=np.float32).reshape(C, HW)
                ),
                "mask": np.ascontiguousarray(
                    np.asarray(src_mask[n], dtype=np.float32).reshape(HW)
                ),
                "wT": wT,
            }
        )
    return maps


def _install_ntff_hook():
    """The agent image's antenv lacks axon_hooks; recreate it so
    run_bass_kernel_spmd(trace=True) can capture NTFF profiles."""
    import sys
    import types

    if "antenv.axon_hooks" in sys.modules:
        return
    import antenv
    from trn_agent_boot.trn_boot import _ntff_profile_via_ctypes

    hook = _ntff_profile_via_ctypes("/opt/axon/libaxon_pjrt.so")
    mod = types.ModuleType("antenv.axon_hooks")
    mod._hook = hook
    mod.set_axon_ntff_profile_hook = lambda h: setattr(mod, "_hook", h)
    mod.get_axon_ntff_profile_hook = lambda: mod._hook
    sys.modules["antenv.axon_hooks"] = mod
    antenv.axon_hooks = mod


def run(src_mask, src_feature, ref_feature, conv_w, trace=False):
    """Run on 8 NeuronCores. Returns (output [N,2C,H,W], BassKernelResults)."""
    n_batch, c, h, w = src_feature.shape
    if trace:
        _install_ntff_hook()
    nc = _get_nc()
    maps = _in_maps(src_mask, src_feature, ref_feature, conv_w)
    res = bass_utils.run_bass_kernel_spmd(
        nc, maps, core_ids=list(range(NCORES)), trace=trace
    )
    out = np.stack([r["out"] for r in res.results], axis=0)
    return out.reshape(n_batch, 2 * c, h, w).astype(np.float32), res


def kernel(src_mask, src_feature, ref_feature, conv_w):
    out, _ = run(src_mask, src_feature, ref_feature, conv_w)
    return out


# revision 22
# speedup vs baseline: 1.6098x; 1.0035x over previous
"""Trainium2 Bass kernel for ExampleGuidedAttention (N=8, C=256, H=W=64).

Data-parallel over batch N across 8 NeuronCores; each core computes one
batch element's full guided attention.

Algorithm notes (per core):
  q = conv_w @ src_pix                      [64, 4096]   (PE, bf16)
  S^T[j,i] = sum_o q[o,j] q[o,i]            (PE, bf16; S symmetric; two
             j-blocks packed in the 128x128 array via tile_position)
  F[j,i] = exp(S^T[j,i] - 64 + 20*ln2)      (ACT; global shift keeps the
             fp32 exp in range; the 2^20 factor cancels against 1/Z)
  Per column-slice s (512 pixels) the F tiles span ALL j, so
  Z[i] = sum_j F[j,i] is computed per-slice with ones-vector matmuls on
  the PE, and each slice normalizes + blends + stores immediately --
  no full-image unnormalized buffer and no end-of-kernel fixup tail.

  Off-diagonal j-block tiles (28 of 32 per slice) are stored in fp8-e4m3
  and applied with DoubleRow matmuls (2 j-blocks per pass, 2x PE rate).
  The 4 diagonal-crossing tiles stay bf16 so the dominant near-diagonal
  attention terms keep full precision; all tiles share the 2^20 scale so
  they accumulate consistently in PSUM and the scale cancels in 1/Z.

  out = [ (1-m)*ref_att*invZ + m*ref ; src_att*invZ ]

The issue order software-pipelines slice s's scores/exp chain (ACT
bound, single PSUM group buffer) against slice s-1's apply matmuls so
the PE never stalls on the exp chain.
"""

import math

import numpy as np

import concourse.bass as bass
import concourse.mybir as mybir
import concourse.tile as tile
from concourse import bacc, bass_utils
from concourse.bass import ts
from concourse.alu_op_type import AluOpType

P = 128
C = 256          # feature channels
CQ = 64          # query channels
HW = 4096        # pixels per image
NB = HW // P     # 32 pixel blocks (contraction chunks)
SLICE = 512
NS = HW // SLICE  # 8 output column slices
NG = NB // 4      # 8 score groups of 4 j-blocks per slice
NCORES = 8
K_SCALE = 13.0    # F scaled by 2^13: off-diag fp8 overflow headroom to S=70

F32 = mybir.dt.float32
BF16 = mybir.dt.bfloat16
F8 = mybir.dt.float8e4
EXP = mybir.ActivationFunctionType.Exp
DR = mybir.MatmulPerfMode.DoubleRow


def _build_body(tc, src, ref, mask, wT, out):
    nc = tc.nc
    src_r = src.ap().rearrange("(ci p) j -> p ci j", p=P)   # [128, 2, 4096]
    ref_r = ref.ap().rearrange("(ci p) j -> p ci j", p=P)
    wT_r = wT.ap().rearrange("(ci p) o -> p ci o", p=P)     # [128, 2, 64]
    out_r = out.ap().rearrange("(cb p) j -> cb p j", p=P)   # [4, 128, 4096]

    with (
        tc.tile_pool(name="persist", bufs=1) as persist,
        tc.tile_pool(name="ps_sc", bufs=1, space="PSUM") as ps_sc,
        tc.tile_pool(name="ps_ap", bufs=2, space="PSUM") as ps_ap,
        tc.tile_pool(name="ps_z", bufs=1, space="PSUM") as ps_z,
        tc.tile_pool(name="dram", bufs=1, space="DRAM") as dram,
    ):
        refb_h = [
            persist.tile([P, 2, HW // 2], BF16, name=f"refb{h}")
            for h in range(2)
        ]
        q2 = persist.tile([P, HW], BF16)
        pixT_bf = persist.tile([P, NB, 2 * C], BF16)   # [src 256 | ref 256]
        pixT_f8 = persist.tile([P, NB, 2 * C], F8)
        wT_sb = persist.tile([P, 2, CQ], BF16)
        mask_rep = persist.tile([P, HW], BF16)
        omask_rep = persist.tile([P, HW], BF16)        # 1 - mask
        mref = persist.tile([P, 2, HW], BF16)          # mask * ref
        exp_bias = persist.tile([P, 1], F32)
        ones8 = persist.tile([P, 2, 16], F8)  # 16B k-pair stride for dual-fp8 ldweights
        ones_bf = persist.tile([P, 2], BF16)
        warm_sb = persist.tile([P, SLICE], BF16)
        zrow = dram.tile([NS, SLICE], F32)
        nc.vector.memset(exp_bias, -64.0 + K_SCALE * math.log(2.0))
        nc.vector.memset(ones8, 1.0)
        nc.vector.memset(ones_bf, 1.0)
        nc.vector.memset(warm_sb, 0.0)

        nc.sync.dma_start(out=wT_sb, in_=wT_r)
        for s in range(NS):
            nc.gpsimd.dma_start(
                out=mask_rep[:, ts(s, SLICE)],
                in_=mask.ap()[ts(s, SLICE)].partition_broadcast(P),
            )

        with tc.tile_pool(name="early", bufs=4) as early, \
             tc.tile_pool(name="early1", bufs=1) as early1:
            # PE warmup: back-to-back matmuls on zeroed data keep the HAM
            # clock gate at 8/8 (2.4 GHz) while input DMAs stream in.
            warm_ps = ps_sc.tile([P, 4, SLICE], F32, name="pss", tag="pss")
            for r in range(20):
                nc.tensor.matmul(
                    warm_ps[:, r % 4, :], warm_sb[:, 0:P], warm_sb,
                    start=True, stop=True,
                )
            # warm the ACT exp table so slice 0 doesn't pay the table load
            nc.scalar.activation(
                out=warm_sb[:, 0:1], in_=warm_sb[:, 0:1], func=EXP,
                bias=exp_bias,
            )

            srcb_h = [
                early1.tile([P, 2, HW // 2], BF16, name=f"srcb{h}")
                for h in range(2)
            ]
            # fp32 inputs via fast hardware DMA.  src casts on DVE (conv ->
            # q gates everything); ref casts on ACT so the DVE can proceed
            # to q2 / fp8 work without head-of-line blocking.
            # src in 8 0.5MB chunks: each dma_start lands on its own
            # hardware queue (~85GB/s each).  ref is issued on sync AFTER
            # the src transposes so src gets the full HBM read bandwidth
            # first (conv -> q -> scores gates everything; ref is needed
            # ~30us later).
            src_stages, ref_stages = [], []
            for c in range(NS):
                st = early.tile([P, 2, SLICE], F32, name="stage", tag="st")
                (nc.sync if c % 2 == 0 else nc.scalar).dma_start(
                    out=st, in_=src_r[:, :, ts(c, SLICE)]
                )
                src_stages.append(st)
            for c in range(NS):
                nc.vector.tensor_copy(
                    out=srcb_h[c // 4][:, :, ts(c % 4, SLICE)],
                    in_=src_stages[c],
                )
            # 1x1 conv: q = wT.T @ src_pix; q into both partition halves.
            # 3 warm matmuls between slices keep the PE (and HAM clock)
            # busy while the next slice's DMA+cast lands.
            for s in range(NS):
                sl = ts(s, SLICE)
                psq = ps_z.tile([CQ, SLICE], F32, name="psz", tag="psz")
                for ci in range(2):
                    nc.tensor.matmul(
                        psq,
                        wT_sb[:, ci, :],
                        srcb_h[s // 4][:, ci, ts(s % 4, SLICE)],
                        start=(ci == 0),
                        stop=(ci == 1),
                    )
                for r in range(3):
                    nc.tensor.matmul(
                        warm_ps[:, r, :], warm_sb[:, 0:P], warm_sb,
                        start=True, stop=True,
                    )
                nc.vector.tensor_copy(out=q2[0:CQ, sl], in_=psq)
                nc.vector.tensor_copy(out=q2[CQ:P, sl], in_=psq)
            # XBAR transposes: pixT[p, b, c] = pix[c, b*128+p]; src halves
            # first (the j-half split lets each start as soon as half the
            # casts have landed), then ref stages + casts + ref transposes.
            for h in range(2):
                bh = slice(h * (NB // 2), (h + 1) * (NB // 2))
                for ci in range(2):
                    nc.sync.dma_start_transpose(
                        out=pixT_bf[:, bh, slice(ci * P, (ci + 1) * P)],
                        in_=srcb_h[h][:, ci, :],
                    )
            for b in range(0, NB, 4):
                nc.vector.tensor_copy(
                    out=pixT_f8[:, b : b + 4, 0:C],
                    in_=pixT_bf[:, b : b + 4, 0:C],
                )
            for c in range(NS):
                st = early.tile([P, 2, SLICE], F32, name="rstage", tag="rst")
                nc.sync.dma_start(out=st, in_=ref_r[:, :, ts(c, SLICE)])
                ref_stages.append(st)
            for c in range(NS):
                nc.vector.tensor_copy(
                    out=refb_h[c // 4][:, :, ts(c % 4, SLICE)],
                    in_=ref_stages[c],
                )
            for h in range(2):
                bh = slice(h * (NB // 2), (h + 1) * (NB // 2))
                for ci in range(2):
                    nc.sync.dma_start_transpose(
                        out=pixT_bf[:, bh, slice(C + ci * P, C + (ci + 1) * P)],
                        in_=refb_h[h][:, ci, :],
                    )
            for b in range(0, NB, 4):
                nc.vector.tensor_copy(
                    out=pixT_f8[:, b : b + 4, C : 2 * C],
                    in_=pixT_bf[:, b : b + 4, C : 2 * C],
                )
            # blend precomputes: 1-m and m*ref
            nc.vector.tensor_scalar(
                out=omask_rep, in0=mask_rep, scalar1=-1.0, scalar2=1.0,
                op0=AluOpType.mult, op1=AluOpType.add,
            )
            for ci in range(2):
                for h in range(2):
                    jh = ts(h, HW // 2)
                    nc.vector.tensor_mul(
                        mref[:, ci, jh], mask_rep[:, jh], refb_h[h][:, ci, :]
                    )

        def f8slot(s, jb):
            return jb if jb < 4 * s else jb - 4

        with tc.tile_pool(name="fbuf", bufs=2) as fbuf, \
             tc.tile_pool(name="obuf", bufs=3) as obuf, \
             tc.tile_pool(name="zbuf", bufs=2) as zbuf:

            def emit_scores_group(s, g, f8, fbf):
                sl = ts(s, SLICE)
                pss = ps_sc.tile([P, 4, SLICE], F32, name="pss", tag="pss")
                for jp in range(2):
                    jb0, jb1 = 4 * g + 2 * jp, 4 * g + 2 * jp + 1
                    nc.tensor.matmul(
                        pss[:, 2 * jp, :], q2[0:CQ, ts(jb0, P)], q2[0:CQ, sl],
                        start=True, stop=True, tile_position=(0, 0),
                    )
                    nc.tensor.matmul(
                        pss[:, 2 * jp + 1, :], q2[CQ:P, ts(jb1, P)],
                        q2[CQ:P, sl],
                        start=True, stop=True, tile_position=(CQ, 0),
                    )
                if g == s:
                    nc.scalar.activation(
                        out=fbf, in_=pss, func=EXP, bias=exp_bias
                    )
                else:
                    fs = f8slot(s, 4 * g)
                    nc.scalar.activation(
                        out=f8[:, fs : fs + 4, :], in_=pss, func=EXP,
                        bias=exp_bias,
                    )

            def emit_z(s, f8, fbf):
                zps = ps_z.tile([2, SLICE], F32, name="psz", tag="psz")
                n_z = 0
                for g in range(NG):
                    if g == s:
                        for r in range(4):
                            nc.tensor.matmul(
                                zps, ones_bf, fbf[:, r, :],
                                start=(n_z == 0), stop=(n_z == 17),
                            )
                            n_z += 1
                    else:
                        fs = f8slot(s, 4 * g)
                        for r in range(2):
                            nc.tensor.matmul(
                                zps, ones8[:, :, 0:2],
                                f8[:, fs + 2 * r : fs + 2 * r + 2, :],
                                start=(n_z == 0), stop=(n_z == 17),
                                perf_mode=DR,
                            )
                            n_z += 1
                zinv_row = zbuf.tile([1, SLICE], F32, name="zr", tag="zr")
                invz_rep = zbuf.tile([P, SLICE], F32, name="zrep", tag="zrep")
                nc.vector.reciprocal_approx_fast(out=zinv_row, in_=zps[0:1, :])
                nc.gpsimd.dma_start(out=zrow[s], in_=zinv_row)
                nc.gpsimd.dma_start(
                    out=invz_rep, in_=zrow[s].partition_broadcast(P)
                )
                return invz_rep

            def emit_warm_chunk():
                wz = ps_z.tile([2, SLICE], F32, name="psz", tag="psz")
                for r in range(9):
                    nc.tensor.matmul(
                        wz, warm_sb[:, 0:2], warm_sb,
                        start=(r == 0), stop=(r == 8),
                    )

            def build_mm_list(s):
                """Ordered apply matmuls: one wave per output channel block
                (cb), each accumulating all 32 j-blocks into a 1-bank PSUM
                tile; start/stop flags per wave."""
                lst = []
                for cb in range(4):
                    blk = []
                    for g in range(NG):
                        if g == s:
                            for r in range(4):
                                blk.append([cb, g, r, False, 0, 0])
                        else:
                            for r in range(2):
                                blk.append([cb, g, r, True, 0, 0])
                    blk[0][4] = 1
                    blk[-1][5] = 1
                    lst.extend(blk)
                return lst

            def emit_apply_mm(ctx, e):
                cb, g, r, is_dr, st, sp = e
                s, f8, fbf, pso = ctx["s"], ctx["f8"], ctx["fbf"], ctx["pso"]
                cs = slice(cb * P, (cb + 1) * P)
                if is_dr:
                    fs = f8slot(s, 4 * g)
                    jb = 4 * g + 2 * r
                    nc.tensor.matmul(
                        pso, pixT_f8[:, jb : jb + 2, cs],
                        f8[:, fs + 2 * r : fs + 2 * r + 2, :],
                        start=bool(st), stop=bool(sp), perf_mode=DR,
                    )
                else:
                    jb = 4 * g + r
                    nc.tensor.matmul(
                        pso, pixT_bf[:, jb, cs], fbf[:, r, :],
                        start=bool(st), stop=bool(sp),
                    )

            def emit_finalize_wave(ctx, cb):
                s, invz_rep = ctx["s"], ctx["invz"]
                sl = ts(s, SLICE)
                outb = ctx["outb"]
                scr = obuf.tile([P, SLICE], F32, name="scr", tag="scr")
                nc.vector.tensor_copy(out=scr, in_=ctx["pso"])
                if cb < 2:
                    # src_att * invz -> out rows 256..512
                    nc.vector.tensor_mul(outb[:, 2 + cb, :], scr, invz_rep)
                else:
                    # flow = ref_att*invz*(1-m) + m*ref -> out rows 0..256
                    if cb == 2:
                        a_s = obuf.tile([P, SLICE], F32, name="a_s", tag="a_s")
                        nc.vector.tensor_mul(
                            a_s, omask_rep[:, sl], invz_rep
                        )
                        ctx["a_s"] = a_s
                    nc.vector.tensor_mul(
                        outb[:, cb - 2, :], scr, ctx["a_s"]
                    )
                    nc.vector.tensor_add(
                        outb[:, cb - 2, :], outb[:, cb - 2, :],
                        mref[:, cb - 2, sl],
                    )
                if cb == 3:
                    oq = [nc.sync, nc.gpsimd]
                    for k in range(4):
                        oq[(s + k) % 2].dma_start(
                            out=out_r[k, :, sl], in_=outb[:, k, :]
                        )

            prev = None
            for s in range(NS):
                f8 = fbuf.tile([P, NB - 4, SLICE], F8, name="f8", tag="f8")
                fbf = fbuf.tile([P, 4, SLICE], BF16, name="fbf", tag="fbf")
                ctx = {"s": s, "f8": f8, "fbf": fbf}
                # interleave this slice's scores/exp chain with the previous
                # slice's apply matmuls in chunks of 9 per score group, so
                # the PE always has work while the exp chain serializes on
                # its single PSUM group buffer.
                for g in range(NG):
                    emit_scores_group(s, g, f8, fbf)
                    if prev is not None:
                        if g == 0:
                            prev["outb"] = obuf.tile(
                                [P, 4, SLICE], F32, name="outb", tag="outb"
                            )
                        if g % 2 == 0:
                            prev["pso"] = ps_ap.tile(
                                [P, SLICE], F32, name="psa", tag="psa"
                            )
                        for e in prev["mm"][9 * g : 9 * g + 9]:
                            emit_apply_mm(prev, e)
                        if g % 2 == 1:
                            emit_finalize_wave(prev, g // 2)
                    elif g < 7:
                        emit_warm_chunk()
                ctx["invz"] = emit_z(s, f8, fbf)
                ctx["mm"] = build_mm_list(s)
                prev = ctx
            # drain: last slice's apply + finalize
            prev["outb"] = obuf.tile([P, 4, SLICE], F32, name="outb", tag="outb")
            for cb in range(4):
                prev["pso"] = ps_ap.tile([P, SLICE], F32, name="psa", tag="psa")
                for e in prev["mm"][18 * cb : 18 * cb + 18]:
                    emit_apply_mm(prev, e)
                emit_finalize_wave(prev, cb)


def build():
    nc = bacc.Bacc(
        "TRN2",
        target_bir_lowering=False,
        debug=False,
        enable_asserts=False,
        num_devices=NCORES,
    )
    src = nc.dram_tensor("src", (C, HW), F32, kind="ExternalInput")
    ref = nc.dram_tensor("ref", (C, HW), F32, kind="ExternalInput")
    mask = nc.dram_tensor("mask", (HW,), F32, kind="ExternalInput")
    wT = nc.dram_tensor("wT", (C, CQ), BF16, kind="ExternalInput")
    out = nc.dram_tensor("out", (2 * C, HW), F32, kind="ExternalOutput")
    with tile.TileContext(nc) as tc:
        _build_body(tc, src, ref, mask, wT, out)
    nc.compile()
    return nc


_CACHE = {}


def _get_nc():
    if "nc" not in _CACHE:
        _CACHE["nc"] = build()
    return _CACHE["nc"]


def _in_maps(src_mask, src_feature, ref_feature, conv_w):
    import ml_dtypes

    n_batch = src_feature.shape[0]
    wT = np.ascontiguousarray(
        np.asarray(conv_w, dtype=np.float32).T.astype(ml_dtypes.bfloat16)
    )
    maps = []
    for n in range(n_batch):
        maps.append(
            {
                "src": np.ascontiguousarray(
                    np.asarray(src_feature[n], dtype=np.float32).reshape(C, HW)
                ),
                "ref": np.ascontiguousarray(
                    np.asarray(ref_feature[n], dtype=np.float32).reshape(C, HW)
                ),
                "mask": np.ascontiguousarray(
                    np.asarray(src_mask[n], dtype=np.float32).reshape(HW)
                ),
                "wT": wT,
            }
        )
    return maps


def _install_ntff_hook():
    """The agent image's antenv lacks axon_hooks; recreate it so
    run_bass_kernel_spmd(trace=True) can capture NTFF profiles."""
    import sys
    import types

    if "antenv.axon_hooks" in sys.modules:
        return
    import antenv
    from trn_agent_boot.trn_boot import _ntff_profile_via_ctypes

    hook = _ntff_profile_via_ctypes("/opt/axon/libaxon_pjrt.so")
    mod = types.ModuleType("antenv.axon_hooks")
    mod._hook = hook
    mod.set_axon_ntff_profile_hook = lambda h: setattr(mod, "_hook", h)
    mod.get_axon_ntff_profile_hook = lambda: mod._hook
    sys.modules["antenv.axon_hooks"] = mod
    antenv.axon_hooks = mod


def run(src_mask, src_feature, ref_feature, conv_w, trace=False):
    """Run on 8 NeuronCores. Returns (output [N,2C,H,W], BassKernelResults)."""
    n_batch, c, h, w = src_feature.shape
    if trace:
        _install_ntff_hook()
    nc = _get_nc()
    maps = _in_maps(src_mask, src_feature, ref_feature, conv_w)
    res = bass_utils.run_bass_kernel_spmd(
        nc, maps, core_ids=list(range(NCORES)), trace=trace
    )
    out = np.stack([r["out"] for r in res.results], axis=0)
    return out.reshape(n_batch, 2 * c, h, w).astype(np.float32), res


def kernel(src_mask, src_feature, ref_feature, conv_w):
    out, _ = run(src_mask, src_feature, ref_feature, conv_w)
    return out


# revision 23
# speedup vs baseline: 1.6403x; 1.0190x over previous
"""Trainium2 Bass kernel for ExampleGuidedAttention (N=8, C=256, H=W=64).

Data-parallel over batch N across 8 NeuronCores; each core computes one
batch element's full guided attention.

Algorithm notes (per core):
  q = conv_w @ src_pix                      [64, 4096]   (PE, bf16)
  S^T[j,i] = sum_o q[o,j] q[o,i]            (PE, bf16; S symmetric; two
             j-blocks packed in the 128x128 array via tile_position)
  F[j,i] = exp(S^T[j,i] - 64 + 20*ln2)      (ACT; global shift keeps the
             fp32 exp in range; the 2^20 factor cancels against 1/Z)
  Per column-slice s (512 pixels) the F tiles span ALL j, so
  Z[i] = sum_j F[j,i] is computed per-slice with ones-vector matmuls on
  the PE, and each slice normalizes + blends + stores immediately --
  no full-image unnormalized buffer and no end-of-kernel fixup tail.

  Off-diagonal j-block tiles (28 of 32 per slice) are stored in fp8-e4m3
  and applied with DoubleRow matmuls (2 j-blocks per pass, 2x PE rate).
  The 4 diagonal-crossing tiles stay bf16 so the dominant near-diagonal
  attention terms keep full precision; all tiles share the 2^20 scale so
  they accumulate consistently in PSUM and the scale cancels in 1/Z.

  out = [ (1-m)*ref_att*invZ + m*ref ; src_att*invZ ]

The issue order software-pipelines slice s's scores/exp chain (ACT
bound, single PSUM group buffer) against slice s-1's apply matmuls so
the PE never stalls on the exp chain.
"""

import math

import numpy as np

import concourse.bass as bass
import concourse.mybir as mybir
import concourse.tile as tile
from concourse import bacc, bass_utils
from concourse.bass import ts
from concourse.alu_op_type import AluOpType

P = 128
C = 256          # feature channels
CQ = 64          # query channels
HW = 4096        # pixels per image
NB = HW // P     # 32 pixel blocks (contraction chunks)
SLICE = 512
NS = HW // SLICE  # 8 output column slices
NG = NB // 4      # 8 score groups of 4 j-blocks per slice
NCORES = 8
K_SCALE = 13.0    # F scaled by 2^13: off-diag fp8 overflow headroom to S=70

F32 = mybir.dt.float32
BF16 = mybir.dt.bfloat16
F8 = mybir.dt.float8e4
EXP = mybir.ActivationFunctionType.Exp
DR = mybir.MatmulPerfMode.DoubleRow


def _build_body(tc, src, ref, mask, wT, out):
    nc = tc.nc
    src_r = src.ap().rearrange("(ci p) j -> p ci j", p=P)   # [128, 2, 4096]
    ref_r = ref.ap().rearrange("(ci p) j -> p ci j", p=P)
    wT_r = wT.ap().rearrange("(ci p) o -> p ci o", p=P)     # [128, 2, 128]
    out_r = out.ap().rearrange("(cb p) j -> cb p j", p=P)   # [4, 128, 4096]

    with (
        tc.tile_pool(name="persist", bufs=1) as persist,
        tc.tile_pool(name="ps_sc", bufs=1, space="PSUM") as ps_sc,
        tc.tile_pool(name="ps_ap", bufs=2, space="PSUM") as ps_ap,
        tc.tile_pool(name="ps_z", bufs=1, space="PSUM") as ps_z,
        tc.tile_pool(name="dram", bufs=1, space="DRAM") as dram,
    ):
        refb_h = [
            persist.tile([P, 2, HW // 2], BF16, name=f"refb{h}")
            for h in range(2)
        ]
        q2 = persist.tile([P, HW], BF16)
        pixT_bf = persist.tile([P, NB, 2 * C], BF16)   # [src 256 | ref 256]
        pixT_f8 = persist.tile([P, NB, 2 * C], F8)
        wT_sb = persist.tile([P, 2, 2 * CQ], BF16)
        mask_rep = persist.tile([P, HW], BF16)
        omask_rep = persist.tile([P, HW], BF16)        # 1 - mask
        mref = persist.tile([P, 2, HW], BF16)          # mask * ref
        exp_bias = persist.tile([P, 1], F32)
        ones8 = persist.tile([P, 2, 16], F8)  # 16B k-pair stride for dual-fp8 ldweights
        ones_bf = persist.tile([P, 2], BF16)
        warm_sb = persist.tile([P, SLICE], BF16)
        zrow = dram.tile([NS, SLICE], F32)
        nc.vector.memset(exp_bias, -64.0 + K_SCALE * math.log(2.0))
        nc.vector.memset(ones8, 1.0)
        nc.vector.memset(ones_bf, 1.0)
        nc.vector.memset(warm_sb, 0.0)

        nc.sync.dma_start(out=wT_sb, in_=wT_r)
        for s in range(NS):
            nc.gpsimd.dma_start(
                out=mask_rep[:, ts(s, SLICE)],
                in_=mask.ap()[ts(s, SLICE)].partition_broadcast(P),
            )

        with tc.tile_pool(name="early", bufs=4) as early, \
             tc.tile_pool(name="early1", bufs=1) as early1:
            # PE warmup: back-to-back matmuls on zeroed data keep the HAM
            # clock gate at 8/8 (2.4 GHz) while input DMAs stream in.
            warm_ps = ps_sc.tile([P, 4, SLICE], F32, name="pss", tag="pss")
            for r in range(20):
                nc.tensor.matmul(
                    warm_ps[:, r % 4, :], warm_sb[:, 0:P], warm_sb,
                    start=True, stop=True,
                )
            # warm the ACT exp table so slice 0 doesn't pay the table load
            nc.scalar.activation(
                out=warm_sb[:, 0:1], in_=warm_sb[:, 0:1], func=EXP,
                bias=exp_bias,
            )

            srcb_h = [
                early1.tile([P, 2, HW // 2], BF16, name=f"srcb{h}")
                for h in range(2)
            ]
            # fp32 inputs via fast hardware DMA.  src casts on DVE (conv ->
            # q gates everything); ref casts on ACT so the DVE can proceed
            # to q2 / fp8 work without head-of-line blocking.
            # src in 8 0.5MB chunks: each dma_start lands on its own
            # hardware queue (~85GB/s each).  ref is issued on sync AFTER
            # the src transposes so src gets the full HBM read bandwidth
            # first (conv -> q -> scores gates everything; ref is needed
            # ~30us later).
            src_stages, ref_stages = [], []
            for c in range(NS):
                st = early.tile([P, 2, SLICE], F32, name="stage", tag="st")
                (nc.sync if c % 2 == 0 else nc.scalar).dma_start(
                    out=st, in_=src_r[:, :, ts(c, SLICE)]
                )
                src_stages.append(st)
            for c in range(NS):
                nc.vector.tensor_copy(
                    out=srcb_h[c // 4][:, :, ts(c % 4, SLICE)],
                    in_=src_stages[c],
                )
            # 1x1 conv: q = wT.T @ src_pix; q into both partition halves.
            # 3 warm matmuls between slices keep the PE (and HAM clock)
            # busy while the next slice's DMA+cast lands.
            for s in range(NS):
                sl = ts(s, SLICE)
                psq = ps_z.tile([P, SLICE], F32, name="psz", tag="psz")
                for ci in range(2):
                    nc.tensor.matmul(
                        psq,
                        wT_sb[:, ci, :],
                        srcb_h[s // 4][:, ci, ts(s % 4, SLICE)],
                        start=(ci == 0),
                        stop=(ci == 1),
                    )
                for r in range(3):
                    nc.tensor.matmul(
                        warm_ps[:, r, :], warm_sb[:, 0:P], warm_sb,
                        start=True, stop=True,
                    )
                nc.vector.tensor_copy(out=q2[:, sl], in_=psq)
            # XBAR transposes: pixT[p, b, c] = pix[c, b*128+p]; src halves
            # first (the j-half split lets each start as soon as half the
            # casts have landed), then ref stages + casts + ref transposes.
            for h in range(2):
                bh = slice(h * (NB // 2), (h + 1) * (NB // 2))
                for ci in range(2):
                    nc.sync.dma_start_transpose(
                        out=pixT_bf[:, bh, slice(ci * P, (ci + 1) * P)],
                        in_=srcb_h[h][:, ci, :],
                    )
            for b in range(0, NB, 4):
                nc.vector.tensor_copy(
                    out=pixT_f8[:, b : b + 4, 0:C],
                    in_=pixT_bf[:, b : b + 4, 0:C],
                )
            for c in range(NS):
                st = early.tile([P, 2, SLICE], F32, name="rstage", tag="rst")
                nc.sync.dma_start(out=st, in_=ref_r[:, :, ts(c, SLICE)])
                ref_stages.append(st)
            for c in range(NS):
                nc.vector.tensor_copy(
                    out=refb_h[c // 4][:, :, ts(c % 4, SLICE)],
                    in_=ref_stages[c],
                )
            for h in range(2):
                bh = slice(h * (NB // 2), (h + 1) * (NB // 2))
                for ci in range(2):
                    nc.sync.dma_start_transpose(
                        out=pixT_bf[:, bh, slice(C + ci * P, C + (ci + 1) * P)],
                        in_=refb_h[h][:, ci, :],
                    )
            for b in range(0, NB, 4):
                nc.vector.tensor_copy(
                    out=pixT_f8[:, b : b + 4, C : 2 * C],
                    in_=pixT_bf[:, b : b + 4, C : 2 * C],
                )
            # blend precomputes: 1-m and m*ref
            nc.vector.tensor_scalar(
                out=omask_rep, in0=mask_rep, scalar1=-1.0, scalar2=1.0,
                op0=AluOpType.mult, op1=AluOpType.add,
            )
            for ci in range(2):
                for h in range(2):
                    jh = ts(h, HW // 2)
                    nc.vector.tensor_mul(
                        mref[:, ci, jh], mask_rep[:, jh], refb_h[h][:, ci, :]
                    )

        def f8slot(s, jb):
            return jb if jb < 4 * s else jb - 4

        with tc.tile_pool(name="fbuf", bufs=2) as fbuf, \
             tc.tile_pool(name="obuf", bufs=3) as obuf, \
             tc.tile_pool(name="zbuf", bufs=2) as zbuf:

            def emit_scores_group(s, g, f8, fbf):
                sl = ts(s, SLICE)
                pss = ps_sc.tile([P, 4, SLICE], F32, name="pss", tag="pss")
                for jp in range(2):
                    jb0, jb1 = 4 * g + 2 * jp, 4 * g + 2 * jp + 1
                    nc.tensor.matmul(
                        pss[:, 2 * jp, :], q2[0:CQ, ts(jb0, P)], q2[0:CQ, sl],
                        start=True, stop=True, tile_position=(0, 0),
                    )
                    nc.tensor.matmul(
                        pss[:, 2 * jp + 1, :], q2[CQ:P, ts(jb1, P)],
                        q2[CQ:P, sl],
                        start=True, stop=True, tile_position=(CQ, 0),
                    )
                if g == s:
                    nc.scalar.activation(
                        out=fbf, in_=pss, func=EXP, bias=exp_bias
                    )
                else:
                    fs = f8slot(s, 4 * g)
                    nc.scalar.activation(
                        out=f8[:, fs : fs + 4, :], in_=pss, func=EXP,
                        bias=exp_bias,
                    )

            def emit_z(s, f8, fbf):
                zps = ps_z.tile([2, SLICE], F32, name="psz", tag="psz")
                n_z = 0
                for g in range(NG):
                    if g == s:
                        for r in range(4):
                            nc.tensor.matmul(
                                zps, ones_bf, fbf[:, r, :],
                                start=(n_z == 0), stop=(n_z == 17),
                            )
                            n_z += 1
                    else:
                        fs = f8slot(s, 4 * g)
                        for r in range(2):
                            nc.tensor.matmul(
                                zps, ones8[:, :, 0:2],
                                f8[:, fs + 2 * r : fs + 2 * r + 2, :],
                                start=(n_z == 0), stop=(n_z == 17),
                                perf_mode=DR,
                            )
                            n_z += 1
                zinv_row = zbuf.tile([1, SLICE], F32, name="zr", tag="zr")
                invz_rep = zbuf.tile([P, SLICE], F32, name="zrep", tag="zrep")
                nc.vector.reciprocal_approx_fast(out=zinv_row, in_=zps[0:1, :])
                nc.gpsimd.dma_start(out=zrow[s], in_=zinv_row)
                nc.gpsimd.dma_start(
                    out=invz_rep, in_=zrow[s].partition_broadcast(P)
                )
                return invz_rep

            def emit_warm_chunk():
                wz = ps_z.tile([2, SLICE], F32, name="psz", tag="psz")
                for r in range(12):
                    nc.tensor.matmul(
                        wz, warm_sb[:, 0:2], warm_sb,
                        start=(r == 0), stop=(r == 11),
                    )

            def build_mm_list(s):
                """Ordered apply matmuls: one wave per output channel block
                (cb), each accumulating all 32 j-blocks into a 1-bank PSUM
                tile; start/stop flags per wave."""
                lst = []
                for cb in range(4):
                    blk = []
                    for g in range(NG):
                        if g == s:
                            for r in range(4):
                                blk.append([cb, g, r, False, 0, 0])
                        else:
                            for r in range(2):
                                blk.append([cb, g, r, True, 0, 0])
                    blk[0][4] = 1
                    blk[-1][5] = 1
                    lst.extend(blk)
                return lst

            def emit_apply_mm(ctx, e):
                cb, g, r, is_dr, st, sp = e
                s, f8, fbf, pso = ctx["s"], ctx["f8"], ctx["fbf"], ctx["pso"]
                cs = slice(cb * P, (cb + 1) * P)
                if is_dr:
                    fs = f8slot(s, 4 * g)
                    jb = 4 * g + 2 * r
                    nc.tensor.matmul(
                        pso, pixT_f8[:, jb : jb + 2, cs],
                        f8[:, fs + 2 * r : fs + 2 * r + 2, :],
                        start=bool(st), stop=bool(sp), perf_mode=DR,
                    )
                else:
                    jb = 4 * g + r
                    nc.tensor.matmul(
                        pso, pixT_bf[:, jb, cs], fbf[:, r, :],
                        start=bool(st), stop=bool(sp),
                    )

            def emit_finalize_wave(ctx, cb):
                s, invz_rep = ctx["s"], ctx["invz"]
                sl = ts(s, SLICE)
                outb = ctx["outb"]
                scr = obuf.tile([P, SLICE], F32, name="scr", tag="scr")
                nc.vector.tensor_copy(out=scr, in_=ctx["pso"])
                if cb < 2:
                    # src_att * invz -> out rows 256..512
                    nc.vector.tensor_mul(outb[:, 2 + cb, :], scr, invz_rep)
                else:
                    # flow = ref_att*invz*(1-m) + m*ref -> out rows 0..256
                    if cb == 2:
                        a_s = obuf.tile([P, SLICE], F32, name="a_s", tag="a_s")
                        nc.vector.tensor_mul(
                            a_s, omask_rep[:, sl], invz_rep
                        )
                        ctx["a_s"] = a_s
                    nc.vector.tensor_mul(
                        outb[:, cb - 2, :], scr, ctx["a_s"]
                    )
                    nc.vector.tensor_add(
                        outb[:, cb - 2, :], outb[:, cb - 2, :],
                        mref[:, cb - 2, sl],
                    )
                if cb == 3:
                    oq = [nc.sync, nc.gpsimd]
                    for k in range(4):
                        oq[(s + k) % 2].dma_start(
                            out=out_r[k, :, sl], in_=outb[:, k, :]
                        )

            prev = None
            for s in range(NS):
                f8 = fbuf.tile([P, NB - 4, SLICE], F8, name="f8", tag="f8")
                fbf = fbuf.tile([P, 4, SLICE], BF16, name="fbf", tag="fbf")
                ctx = {"s": s, "f8": f8, "fbf": fbf}
                # interleave this slice's scores/exp chain with the previous
                # slice's apply matmuls in chunks of 9 per score group, so
                # the PE always has work while the exp chain serializes on
                # its single PSUM group buffer.
                for g in range(NG):
                    emit_scores_group(s, g, f8, fbf)
                    if prev is not None:
                        if g == 0:
                            prev["outb"] = obuf.tile(
                                [P, 4, SLICE], F32, name="outb", tag="outb"
                            )
                        if g % 2 == 0:
                            prev["pso"] = ps_ap.tile(
                                [P, SLICE], F32, name="psa", tag="psa"
                            )
                        for e in prev["mm"][9 * g : 9 * g + 9]:
                            emit_apply_mm(prev, e)
                        if g % 2 == 1:
                            emit_finalize_wave(prev, g // 2)
                    elif g < 7:
                        emit_warm_chunk()
                ctx["invz"] = emit_z(s, f8, fbf)
                ctx["mm"] = build_mm_list(s)
                prev = ctx
            # drain: last slice's apply + finalize
            prev["outb"] = obuf.tile([P, 4, SLICE], F32, name="outb", tag="outb")
            for cb in range(4):
                prev["pso"] = ps_ap.tile([P, SLICE], F32, name="psa", tag="psa")
                for e in prev["mm"][18 * cb : 18 * cb + 18]:
                    emit_apply_mm(prev, e)
                emit_finalize_wave(prev, cb)


def build():
    nc = bacc.Bacc(
        "TRN2",
        target_bir_lowering=False,
        debug=False,
        enable_asserts=False,
        num_devices=NCORES,
    )
    src = nc.dram_tensor("src", (C, HW), F32, kind="ExternalInput")
    ref = nc.dram_tensor("ref", (C, HW), F32, kind="ExternalInput")
    mask = nc.dram_tensor("mask", (HW,), F32, kind="ExternalInput")
    wT = nc.dram_tensor("wT", (C, 2 * CQ), BF16, kind="ExternalInput")
    out = nc.dram_tensor("out", (2 * C, HW), F32, kind="ExternalOutput")
    with tile.TileContext(nc) as tc:
        _build_body(tc, src, ref, mask, wT, out)
    nc.compile()
    return nc


_CACHE = {}


def _get_nc():
    if "nc" not in _CACHE:
        _CACHE["nc"] = build()
    return _CACHE["nc"]


def _in_maps(src_mask, src_feature, ref_feature, conv_w):
    import ml_dtypes

    n_batch = src_feature.shape[0]
    wT1 = np.asarray(conv_w, dtype=np.float32).T.astype(ml_dtypes.bfloat16)
    # duplicated columns: the conv then writes q into BOTH partition halves
    # of q2 in one matmul (the scores pairs need q at partitions 0-63 and
    # 64-127 for tile_position packing)
    wT = np.ascontiguousarray(np.concatenate([wT1, wT1], axis=1))
    maps = []
    for n in range(n_batch):
        maps.append(
            {
                "src": np.ascontiguousarray(
                    np.asarray(src_feature[n], dtype=np.float32).reshape(C, HW)
                ),
                "ref": np.ascontiguousarray(
                    np.asarray(ref_feature[n], dtype=np.float32).reshape(C, HW)
                ),
                "mask": np.ascontiguousarray(
                    np.asarray(src_mask[n], dtype=np.float32).reshape(HW)
                ),
                "wT": wT,
            }
        )
    return maps


def _install_ntff_hook():
    """The agent image's antenv lacks axon_hooks; recreate it so
    run_bass_kernel_spmd(trace=True) can capture NTFF profiles."""
    import sys
    import types

    if "antenv.axon_hooks" in sys.modules:
        return
    import antenv
    from trn_agent_boot.trn_boot import _ntff_profile_via_ctypes

    hook = _ntff_profile_via_ctypes("/opt/axon/libaxon_pjrt.so")
    mod = types.ModuleType("antenv.axon_hooks")
    mod._hook = hook
    mod.set_axon_ntff_profile_hook = lambda h: setattr(mod, "_hook", h)
    mod.get_axon_ntff_profile_hook = lambda: mod._hook
    sys.modules["antenv.axon_hooks"] = mod
    antenv.axon_hooks = mod


def run(src_mask, src_feature, ref_feature, conv_w, trace=False):
    """Run on 8 NeuronCores. Returns (output [N,2C,H,W], BassKernelResults)."""
    n_batch, c, h, w = src_feature.shape
    if trace:
        _install_ntff_hook()
    nc = _get_nc()
    maps = _in_maps(src_mask, src_feature, ref_feature, conv_w)
    res = bass_utils.run_bass_kernel_spmd(
        nc, maps, core_ids=list(range(NCORES)), trace=trace
    )
    out = np.stack([r["out"] for r in res.results], axis=0)
    return out.reshape(n_batch, 2 * c, h, w).astype(np.float32), res


def kernel(src_mask, src_feature, ref_feature, conv_w):
    out, _ = run(src_mask, src_feature, ref_feature, conv_w)
    return out


# revision 24
# speedup vs baseline: 1.6414x; 1.0007x over previous
"""Trainium2 Bass kernel for ExampleGuidedAttention (N=8, C=256, H=W=64).

Data-parallel over batch N across 8 NeuronCores; each core computes one
batch element's full guided attention.

Algorithm notes (per core):
  q = conv_w @ src_pix                      [64, 4096]   (PE, bf16)
  S^T[j,i] = sum_o q[o,j] q[o,i]            (PE, bf16; S symmetric; two
             j-blocks packed in the 128x128 array via tile_position)
  F[j,i] = exp(S^T[j,i] - 64 + 20*ln2)      (ACT; global shift keeps the
             fp32 exp in range; the 2^20 factor cancels against 1/Z)
  Per column-slice s (512 pixels) the F tiles span ALL j, so
  Z[i] = sum_j F[j,i] is computed per-slice with ones-vector matmuls on
  the PE, and each slice normalizes + blends + stores immediately --
  no full-image unnormalized buffer and no end-of-kernel fixup tail.

  Off-diagonal j-block tiles (28 of 32 per slice) are stored in fp8-e4m3
  and applied with DoubleRow matmuls (2 j-blocks per pass, 2x PE rate).
  The 4 diagonal-crossing tiles stay bf16 so the dominant near-diagonal
  attention terms keep full precision; all tiles share the 2^20 scale so
  they accumulate consistently in PSUM and the scale cancels in 1/Z.

  out = [ (1-m)*ref_att*invZ + m*ref ; src_att*invZ ]

The issue order software-pipelines slice s's scores/exp chain (ACT
bound, single PSUM group buffer) against slice s-1's apply matmuls so
the PE never stalls on the exp chain.
"""

import math

import numpy as np

import concourse.bass as bass
import concourse.mybir as mybir
import concourse.tile as tile
from concourse import bacc, bass_utils
from concourse.bass import ts
from concourse.alu_op_type import AluOpType

P = 128
C = 256          # feature channels
CQ = 64          # query channels
HW = 4096        # pixels per image
NB = HW // P     # 32 pixel blocks (contraction chunks)
SLICE = 512
NS = HW // SLICE  # 8 output column slices
NG = NB // 4      # 8 score groups of 4 j-blocks per slice
NCORES = 8
K_SCALE = 13.0    # F scaled by 2^13: off-diag fp8 overflow headroom to S=70

F32 = mybir.dt.float32
BF16 = mybir.dt.bfloat16
F8 = mybir.dt.float8e4
EXP = mybir.ActivationFunctionType.Exp
DR = mybir.MatmulPerfMode.DoubleRow


def _build_body(tc, src, ref, mask, wT, out):
    nc = tc.nc
    src_r = src.ap().rearrange("(ci p) j -> p ci j", p=P)   # [128, 2, 4096]
    ref_r = ref.ap().rearrange("(ci p) j -> p ci j", p=P)
    wT_r = wT.ap().rearrange("(ci p) o -> p ci o", p=P)     # [128, 2, 128]
    out_r = out.ap().rearrange("(cb p) j -> cb p j", p=P)   # [4, 128, 4096]

    with (
        tc.tile_pool(name="persist", bufs=1) as persist,
        tc.tile_pool(name="ps_sc", bufs=1, space="PSUM") as ps_sc,
        tc.tile_pool(name="ps_ap", bufs=2, space="PSUM") as ps_ap,
        tc.tile_pool(name="ps_z", bufs=1, space="PSUM") as ps_z,
        tc.tile_pool(name="dram", bufs=1, space="DRAM") as dram,
    ):
        refb_h = [
            persist.tile([P, 2, HW // 2], BF16, name=f"refb{h}")
            for h in range(2)
        ]
        q2 = persist.tile([P, HW], BF16)
        pixT_bf = persist.tile([P, NB, 2 * C], BF16)   # [src 256 | ref 256]
        pixT_f8 = persist.tile([P, NB, 2 * C], F8)
        wT_sb = persist.tile([P, 2, 2 * CQ], BF16)
        mask_rep = persist.tile([P, HW], BF16)
        omask_rep = persist.tile([P, HW], BF16)        # 1 - mask
        mref = persist.tile([P, 2, HW], BF16)          # mask * ref
        exp_bias = persist.tile([P, 1], F32)
        ones8 = persist.tile([P, 2, 16], F8)  # 16B k-pair stride for dual-fp8 ldweights
        ones_bf = persist.tile([P, 2], BF16)
        warm_sb = persist.tile([P, SLICE], BF16)
        zrow = dram.tile([NS, SLICE], F32)
        nc.vector.memset(exp_bias, -64.0 + K_SCALE * math.log(2.0))
        nc.vector.memset(ones8, 1.0)
        nc.vector.memset(ones_bf, 1.0)
        nc.vector.memset(warm_sb, 0.0)

        nc.sync.dma_start(out=wT_sb, in_=wT_r)
        for s in range(NS):
            nc.gpsimd.dma_start(
                out=mask_rep[:, ts(s, SLICE)],
                in_=mask.ap()[ts(s, SLICE)].partition_broadcast(P),
            )

        with tc.tile_pool(name="early", bufs=6) as early, \
             tc.tile_pool(name="early1", bufs=1) as early1:
            # PE warmup: back-to-back matmuls on zeroed data keep the HAM
            # clock gate at 8/8 (2.4 GHz) while input DMAs stream in.
            warm_ps = ps_sc.tile([P, 4, SLICE], F32, name="pss", tag="pss")
            for r in range(20):
                nc.tensor.matmul(
                    warm_ps[:, r % 4, :], warm_sb[:, 0:P], warm_sb,
                    start=True, stop=True,
                )
            # warm the ACT exp table so slice 0 doesn't pay the table load
            nc.scalar.activation(
                out=warm_sb[:, 0:1], in_=warm_sb[:, 0:1], func=EXP,
                bias=exp_bias,
            )

            srcb_h = [
                early1.tile([P, 2, HW // 2], BF16, name=f"srcb{h}")
                for h in range(2)
            ]
            # fp32 inputs via fast hardware DMA.  src casts on DVE (conv ->
            # q gates everything); ref casts on ACT so the DVE can proceed
            # to q2 / fp8 work without head-of-line blocking.
            # src in 8 0.5MB chunks: each dma_start lands on its own
            # hardware queue (~85GB/s each).  ref is issued on sync AFTER
            # the src transposes so src gets the full HBM read bandwidth
            # first (conv -> q -> scores gates everything; ref is needed
            # ~30us later).
            src_stages, ref_stages = [], []
            SC = SLICE // 2
            for c in range(16):
                st = early.tile([P, 2, SC], F32, name="stage", tag="st")
                (nc.sync if c % 2 == 0 else nc.scalar).dma_start(
                    out=st, in_=src_r[:, :, ts(c, SC)]
                )
                src_stages.append(st)
            for c in range(16):
                nc.vector.tensor_copy(
                    out=srcb_h[c // 8][:, :, ts(c % 8, SC)],
                    in_=src_stages[c],
                )
            # 1x1 conv: q = wT.T @ src_pix; q into both partition halves.
            # 3 warm matmuls between slices keep the PE (and HAM clock)
            # busy while the next slice's DMA+cast lands.
            for s in range(NS):
                sl = ts(s, SLICE)
                psq = ps_z.tile([P, SLICE], F32, name="psz", tag="psz")
                for ci in range(2):
                    nc.tensor.matmul(
                        psq,
                        wT_sb[:, ci, :],
                        srcb_h[s // 4][:, ci, ts(s % 4, SLICE)],
                        start=(ci == 0),
                        stop=(ci == 1),
                    )
                for r in range(3):
                    nc.tensor.matmul(
                        warm_ps[:, r, :], warm_sb[:, 0:P], warm_sb,
                        start=True, stop=True,
                    )
                nc.vector.tensor_copy(out=q2[:, sl], in_=psq)
            # XBAR transposes: pixT[p, b, c] = pix[c, b*128+p]; src halves
            # first (the j-half split lets each start as soon as half the
            # casts have landed), then ref stages + casts + ref transposes.
            for h in range(2):
                bh = slice(h * (NB // 2), (h + 1) * (NB // 2))
                for ci in range(2):
                    nc.sync.dma_start_transpose(
                        out=pixT_bf[:, bh, slice(ci * P, (ci + 1) * P)],
                        in_=srcb_h[h][:, ci, :],
                    )
            for b in range(0, NB, 4):
                nc.vector.tensor_copy(
                    out=pixT_f8[:, b : b + 4, 0:C],
                    in_=pixT_bf[:, b : b + 4, 0:C],
                )
            for c in range(NS):
                st = early.tile([P, 2, SLICE], F32, name="rstage", tag="rst")
                nc.sync.dma_start(out=st, in_=ref_r[:, :, ts(c, SLICE)])
                ref_stages.append(st)
            for c in range(NS):
                nc.vector.tensor_copy(
                    out=refb_h[c // 4][:, :, ts(c % 4, SLICE)],
                    in_=ref_stages[c],
                )
            for h in range(2):
                bh = slice(h * (NB // 2), (h + 1) * (NB // 2))
                for ci in range(2):
                    nc.sync.dma_start_transpose(
                        out=pixT_bf[:, bh, slice(C + ci * P, C + (ci + 1) * P)],
                        in_=refb_h[h][:, ci, :],
                    )
            for b in range(0, NB, 4):
                nc.vector.tensor_copy(
                    out=pixT_f8[:, b : b + 4, C : 2 * C],
                    in_=pixT_bf[:, b : b + 4, C : 2 * C],
                )
            # blend precomputes: 1-m and m*ref
            nc.vector.tensor_scalar(
                out=omask_rep, in0=mask_rep, scalar1=-1.0, scalar2=1.0,
                op0=AluOpType.mult, op1=AluOpType.add,
            )
            for ci in range(2):
                for h in range(2):
                    jh = ts(h, HW // 2)
                    nc.vector.tensor_mul(
                        mref[:, ci, jh], mask_rep[:, jh], refb_h[h][:, ci, :]
                    )

        def f8slot(s, jb):
            return jb if jb < 4 * s else jb - 4

        with tc.tile_pool(name="fbuf", bufs=2) as fbuf, \
             tc.tile_pool(name="obuf", bufs=3) as obuf, \
             tc.tile_pool(name="zbuf", bufs=2) as zbuf:

            def emit_scores_group(s, g, f8, fbf):
                sl = ts(s, SLICE)
                pss = ps_sc.tile([P, 4, SLICE], F32, name="pss", tag="pss")
                for jp in range(2):
                    jb0, jb1 = 4 * g + 2 * jp, 4 * g + 2 * jp + 1
                    nc.tensor.matmul(
                        pss[:, 2 * jp, :], q2[0:CQ, ts(jb0, P)], q2[0:CQ, sl],
                        start=True, stop=True, tile_position=(0, 0),
                    )
                    nc.tensor.matmul(
                        pss[:, 2 * jp + 1, :], q2[CQ:P, ts(jb1, P)],
                        q2[CQ:P, sl],
                        start=True, stop=True, tile_position=(CQ, 0),
                    )
                if g == s:
                    nc.scalar.activation(
                        out=fbf, in_=pss, func=EXP, bias=exp_bias
                    )
                else:
                    fs = f8slot(s, 4 * g)
                    nc.scalar.activation(
                        out=f8[:, fs : fs + 4, :], in_=pss, func=EXP,
                        bias=exp_bias,
                    )

            def emit_z(s, f8, fbf):
                zps = ps_z.tile([2, SLICE], F32, name="psz", tag="psz")
                n_z = 0
                for g in range(NG):
                    if g == s:
                        for r in range(4):
                            nc.tensor.matmul(
                                zps, ones_bf, fbf[:, r, :],
                                start=(n_z == 0), stop=(n_z == 17),
                            )
                            n_z += 1
                    else:
                        fs = f8slot(s, 4 * g)
                        for r in range(2):
                            nc.tensor.matmul(
                                zps, ones8[:, :, 0:2],
                                f8[:, fs + 2 * r : fs + 2 * r + 2, :],
                                start=(n_z == 0), stop=(n_z == 17),
                                perf_mode=DR,
                            )
                            n_z += 1
                zinv_row = zbuf.tile([1, SLICE], F32, name="zr", tag="zr")
                invz_rep = zbuf.tile([P, SLICE], F32, name="zrep", tag="zrep")
                nc.vector.reciprocal_approx_fast(out=zinv_row, in_=zps[0:1, :])
                nc.gpsimd.dma_start(out=zrow[s], in_=zinv_row)
                nc.gpsimd.dma_start(
                    out=invz_rep, in_=zrow[s].partition_broadcast(P)
                )
                return invz_rep

            def emit_warm_chunk():
                wz = ps_z.tile([2, SLICE], F32, name="psz", tag="psz")
                for r in range(12):
                    nc.tensor.matmul(
                        wz, warm_sb[:, 0:2], warm_sb,
                        start=(r == 0), stop=(r == 11),
                    )

            def build_mm_list(s):
                """Ordered apply matmuls: one wave per output channel block
                (cb), each accumulating all 32 j-blocks into a 1-bank PSUM
                tile; start/stop flags per wave."""
                lst = []
                for cb in range(4):
                    blk = []
                    for g in range(NG):
                        if g == s:
                            for r in range(4):
                                blk.append([cb, g, r, False, 0, 0])
                        else:
                            for r in range(2):
                                blk.append([cb, g, r, True, 0, 0])
                    blk[0][4] = 1
                    blk[-1][5] = 1
                    lst.extend(blk)
                return lst

            def emit_apply_mm(ctx, e):
                cb, g, r, is_dr, st, sp = e
                s, f8, fbf, pso = ctx["s"], ctx["f8"], ctx["fbf"], ctx["pso"]
                cs = slice(cb * P, (cb + 1) * P)
                if is_dr:
                    fs = f8slot(s, 4 * g)
                    jb = 4 * g + 2 * r
                    nc.tensor.matmul(
                        pso, pixT_f8[:, jb : jb + 2, cs],
                        f8[:, fs + 2 * r : fs + 2 * r + 2, :],
                        start=bool(st), stop=bool(sp), perf_mode=DR,
                    )
                else:
                    jb = 4 * g + r
                    nc.tensor.matmul(
                        pso, pixT_bf[:, jb, cs], fbf[:, r, :],
                        start=bool(st), stop=bool(sp),
                    )

            def emit_finalize_wave(ctx, cb):
                s, invz_rep = ctx["s"], ctx["invz"]
                sl = ts(s, SLICE)
                outb = ctx["outb"]
                scr = obuf.tile([P, SLICE], F32, name="scr", tag="scr")
                nc.vector.tensor_copy(out=scr, in_=ctx["pso"])
                if cb < 2:
                    # src_att * invz -> out rows 256..512
                    nc.vector.tensor_mul(outb[:, 2 + cb, :], scr, invz_rep)
                else:
                    # flow = ref_att*invz*(1-m) + m*ref -> out rows 0..256
                    if cb == 2:
                        a_s = obuf.tile([P, SLICE], F32, name="a_s", tag="a_s")
                        nc.vector.tensor_mul(
                            a_s, omask_rep[:, sl], invz_rep
                        )
                        ctx["a_s"] = a_s
                    nc.vector.tensor_mul(
                        outb[:, cb - 2, :], scr, ctx["a_s"]
                    )
                    nc.vector.tensor_add(
                        outb[:, cb - 2, :], outb[:, cb - 2, :],
                        mref[:, cb - 2, sl],
                    )
                if cb == 3:
                    oq = [nc.sync, nc.gpsimd]
                    for k in range(4):
                        oq[(s + k) % 2].dma_start(
                            out=out_r[k, :, sl], in_=outb[:, k, :]
                        )

            prev = None
            for s in range(NS):
                f8 = fbuf.tile([P, NB - 4, SLICE], F8, name="f8", tag="f8")
                fbf = fbuf.tile([P, 4, SLICE], BF16, name="fbf", tag="fbf")
                ctx = {"s": s, "f8": f8, "fbf": fbf}
                # interleave this slice's scores/exp chain with the previous
                # slice's apply matmuls in chunks of 9 per score group, so
                # the PE always has work while the exp chain serializes on
                # its single PSUM group buffer.
                for g in range(NG):
                    emit_scores_group(s, g, f8, fbf)
                    if prev is not None:
                        if g == 0:
                            prev["outb"] = obuf.tile(
                                [P, 4, SLICE], F32, name="outb", tag="outb"
                            )
                        if g % 2 == 0:
                            prev["pso"] = ps_ap.tile(
                                [P, SLICE], F32, name="psa", tag="psa"
                            )
                        for e in prev["mm"][9 * g : 9 * g + 9]:
                            emit_apply_mm(prev, e)
                        if g % 2 == 1:
                            emit_finalize_wave(prev, g // 2)
                    elif g < 7:
                        emit_warm_chunk()
                ctx["invz"] = emit_z(s, f8, fbf)
                ctx["mm"] = build_mm_list(s)
                prev = ctx
            # drain: last slice's apply + finalize
            prev["outb"] = obuf.tile([P, 4, SLICE], F32, name="outb", tag="outb")
            for cb in range(4):
                prev["pso"] = ps_ap.tile([P, SLICE], F32, name="psa", tag="psa")
                for e in prev["mm"][18 * cb : 18 * cb + 18]:
                    emit_apply_mm(prev, e)
                emit_finalize_wave(prev, cb)
            # keep the HAM clock at 8/8 while the last finalize + output
            # DMAs drain (the blend + stores run ~2x slower at half clock)
            for k in range(4):
                emit_warm_chunk()


def build():
    nc = bacc.Bacc(
        "TRN2",
        target_bir_lowering=False,
        debug=False,
        enable_asserts=False,
        num_devices=NCORES,
    )
    src = nc.dram_tensor("src", (C, HW), F32, kind="ExternalInput")
    ref = nc.dram_tensor("ref", (C, HW), F32, kind="ExternalInput")
    mask = nc.dram_tensor("mask", (HW,), F32, kind="ExternalInput")
    wT = nc.dram_tensor("wT", (C, 2 * CQ), BF16, kind="ExternalInput")
    out = nc.dram_tensor("out", (2 * C, HW), F32, kind="ExternalOutput")
    with tile.TileContext(nc) as tc:
        _build_body(tc, src, ref, mask, wT, out)
    nc.compile()
    return nc


_CACHE = {}


def _get_nc():
    if "nc" not in _CACHE:
        _CACHE["nc"] = build()
    return _CACHE["nc"]


def _in_maps(src_mask, src_feature, ref_feature, conv_w):
    import ml_dtypes

    n_batch = src_feature.shape[0]
    wT1 = np.asarray(conv_w, dtype=np.float32).T.astype(ml_dtypes.bfloat16)
    # duplicated columns: the conv then writes q into BOTH partition halves
    # of q2 in one matmul (the scores pairs need q at partitions 0-63 and
    # 64-127 for tile_position packing)
    wT = np.ascontiguousarray(np.concatenate([wT1, wT1], axis=1))
    maps = []
    for n in range(n_batch):
        maps.append(
            {
                "src": np.ascontiguousarray(
                    np.asarray(src_feature[n], dtype=np.float32).reshape(C, HW)
                ),
                "ref": np.ascontiguousarray(
                    np.asarray(ref_feature[n], dtype=np.float32).reshape(C, HW)
                ),
                "mask": np.ascontiguousarray(
                    np.asarray(src_mask[n], dtype=np.float32).reshape(HW)
                ),
                "wT": wT,
            }
        )
    return maps


def _install_ntff_hook():
    """The agent image's antenv lacks axon_hooks; recreate it so
    run_bass_kernel_spmd(trace=True) can capture NTFF profiles."""
    import sys
    import types

    if "antenv.axon_hooks" in sys.modules:
        return
    import antenv
    from trn_agent_boot.trn_boot import _ntff_profile_via_ctypes

    hook = _ntff_profile_via_ctypes("/opt/axon/libaxon_pjrt.so")
    mod = types.ModuleType("antenv.axon_hooks")
    mod._hook = hook
    mod.set_axon_ntff_profile_hook = lambda h: setattr(mod, "_hook", h)
    mod.get_axon_ntff_profile_hook = lambda: mod._hook
    sys.modules["antenv.axon_hooks"] = mod
    antenv.axon_hooks = mod


def run(src_mask, src_feature, ref_feature, conv_w, trace=False):
    """Run on 8 NeuronCores. Returns (output [N,2C,H,W], BassKernelResults)."""
    n_batch, c, h, w = src_feature.shape
    if trace:
        _install_ntff_hook()
    nc = _get_nc()
    maps = _in_maps(src_mask, src_feature, ref_feature, conv_w)
    res = bass_utils.run_bass_kernel_spmd(
        nc, maps, core_ids=list(range(NCORES)), trace=trace
    )
    out = np.stack([r["out"] for r in res.results], axis=0)
    return out.reshape(n_batch, 2 * c, h, w).astype(np.float32), res


def kernel(src_mask, src_feature, ref_feature, conv_w):
    out, _ = run(src_mask, src_feature, ref_feature, conv_w)
    return out
